# revision 2
# baseline (speedup 1.0000x reference)
"""Trainium2 Bass kernel: batched recursive Newton-Euler inverse dynamics
(7-dof serial chain) — data-parallel over 8 NeuronCores.

Per core, the 65536-row shard lives as fp16 planes [128 part, 512 free]
(fp16 halves DVE tensor_tensor time via the 2x_1p perf mode; validated
rel-err ~1.7e-3 vs the 2e-2 gate). Per-link parameters are baked in as
immediate constants. The physics is emitted through a symbolic layer
(Val = a*plane + c) that prunes zeros, folds scales, and chains every
n-term linear combination into n-1 fused scalar_tensor_tensor ops.
Ops are recorded into a tiny IR, dead code is eliminated, and a
HEFT-style list scheduler assigns each op to an engine (DVE / Pool /
ACT) to overlap the three elementwise-capable engines. Trig is computed
once per joint (shared between fwd and bwd passes) with a single range
reduction: s = Sin(z), c = Sin(pi/2 - |z|).
"""

import math
from contextlib import ExitStack

import numpy as np

P = 128
D = 7
N_CORES = 8
BATCH = 524288
SHARD = BATCH // N_CORES      # 65536
FD = SHARD // P               # 512

TWO_PI = 2.0 * math.pi
HALF_PI = math.pi / 2
MAGIC = 12582912.0            # 1.5 * 2**23, fp32 round-to-nearest trick

DT16 = True                   # fp16 planes
USE_GP = True                 # allow Pool (gpsimd) engine for tensor_tensor


# ---------------------------------------------------------------------------
# symbolic value: a * plane + c   (plane None -> pure constant)
# ---------------------------------------------------------------------------
class Val:
    __slots__ = ("pl", "a", "c")

    def __init__(self, pl, a=1.0, c=0.0):
        self.pl = pl
        self.a = float(a)
        self.c = float(c)
        if pl is None:
            self.a = 0.0

    @property
    def is_const(self):
        return self.pl is None or self.a == 0.0


def VC(c):
    return Val(None, 0.0, c)


class Builder:
    """Backend-agnostic emitter. Each primitive is exactly one instruction."""

    def __init__(self):
        self.n_2src = 0
        self.n_1src = 0
        self.n_trig = 0
        self.phase = ""
        self._ones = None
        self._trig = {}

    # ---- primitives (backends) ----
    def p_stt(self, in0, scalar, in1, op1, dest=None, f32=False):
        raise NotImplementedError

    def p_tt(self, in0, in1, op, dest=None):
        raise NotImplementedError

    def p_affine(self, in0, scale, bias, dest=None):
        raise NotImplementedError

    def p_act(self, in0, fname, scale, bias, f32=False):
        raise NotImplementedError

    def p_ones(self):
        raise NotImplementedError

    def inp(self, name, j):
        raise NotImplementedError

    def out_ap(self, j):
        raise NotImplementedError

    def f_ap(self, j, i):
        raise NotImplementedError

    def state_ap(self, j, i):
        raise NotImplementedError

    def plane_key(self, pl):
        return id(pl)

    def same_plane(self, a, b):
        return a is b

    # ---- helpers ----
    def ones(self):
        if self._ones is None:
            self._ones = self.p_ones()
        return self._ones

    def sincos(self, j):
        # one range reduction per joint; cos from |z|: cos z = sin(pi/2 - |z|)
        if j in self._trig:
            return self._trig[j]
        q = self.inp("q", j)
        u = self.p_act(q, "Copy", 1.0 / TWO_PI, MAGIC, f32=True)
        r = self.p_act(u, "Copy", 1.0, -MAGIC, f32=True)
        z = self.p_stt(r, -TWO_PI, q, "add", f32=True)
        s = Val(self.p_act(z, "Sin", 1.0, 0.0))
        a = self.p_act(z, "Abs", 1.0, 0.0, f32=True)
        c = Val(self.p_act(a, "Sin", -1.0, HALF_PI))
        self.n_trig += 2
        self._trig[j] = (s, c)
        return s, c

    def lin(self, vals, coefs, const=0.0, dest=None, exact=False, scale_free=False):
        terms = {}
        c_acc = float(const)
        for v, k in zip(vals, coefs):
            k = float(k)
            if k == 0.0:
                continue
            c_acc += k * v.c
            if v.pl is not None and v.a != 0.0:
                key = self.plane_key(v.pl)
                if key in terms:
                    terms[key][1] += k * v.a
                else:
                    terms[key] = [v.pl, k * v.a]
        tl = [(pl, k) for pl, k in terms.values() if k != 0.0]
        if not tl:
            if dest is not None:
                self.n_1src += 1
                self.p_affine(self.ones(), c_acc, 0.0, dest=dest)
                return Val(dest, 1.0, 0.0)
            return VC(c_acc)
        if c_acc != 0.0:
            tl.append((self.ones(), c_acc))
        if len(tl) == 1:
            pl, k = tl[0]
            if dest is not None:
                self.n_1src += 1
                self.p_affine(pl, k, 0.0, dest=dest)
                return Val(dest, 1.0, 0.0)
            if exact and k != 1.0:
                self.n_1src += 1
                return Val(self.p_affine(pl, k, 0.0), 1.0, 0.0)
            return Val(pl, k, 0.0)
        tl.sort(key=lambda t: abs(t[1]))
        cur_pl, cur_k = tl[0]
        for i in range(1, len(tl)):
            pl_i, k_i = tl[i]
            is_last = i == len(tl) - 1
            use_dest = dest is not None and is_last and (scale_free or k_i == 1.0)
            d = dest if use_dest else None
            self.n_2src += 1
            cur_pl = self.p_stt(cur_pl, cur_k / k_i, pl_i, "add", dest=d)
            cur_k = k_i
        if dest is not None and not self.same_plane(cur_pl, dest):
            self.n_1src += 1
            self.p_affine(cur_pl, cur_k, 0.0, dest=dest)
            return Val(dest, 1.0, 0.0)
        if dest is not None:
            return Val(dest, cur_k if scale_free else 1.0, 0.0)
        if exact and cur_k != 1.0:
            self.n_1src += 1
            return Val(self.p_affine(cur_pl, cur_k, 0.0), 1.0, 0.0)
        return Val(cur_pl, cur_k, 0.0)

    def mov(self, v, dest):
        self.n_1src += 1
        if v.pl is None:
            self.p_affine(self.ones(), v.c, 0.0, dest=dest)
        else:
            self.p_affine(v.pl, v.a, v.c, dest=dest)
        return Val(dest, 1.0, 0.0)

    def mul(self, x, y):
        if x.is_const and y.is_const:
            return VC(x.c * y.c)
        if x.is_const:
            x, y = y, x
        if y.is_const:
            return Val(x.pl, x.a * y.c, x.c * y.c)
        xp, yp = x, y
        if xp.c != 0.0:
            self.n_1src += 1
            xp = Val(self.p_affine(xp.pl, 1.0, xp.c / xp.a), xp.a, 0.0)
        if yp.c != 0.0:
            self.n_1src += 1
            yp = Val(self.p_affine(yp.pl, 1.0, yp.c / yp.a), yp.a, 0.0)
        self.n_2src += 1
        out = self.p_tt(xp.pl, yp.pl, "mult")
        return Val(out, xp.a * yp.a, 0.0)

    def cross(self, u, v):
        out = []
        for i in range(3):
            b, c = (i + 1) % 3, (i + 2) % 3
            m1 = self.mul(u[b], v[c])
            m2 = self.mul(u[c], v[b])
            out.append((m1, m2))
        return out

    def matvec(self, M, v):
        return [self.lin(v, [M[i][0], M[i][1], M[i][2]]) for i in range(3)]

    def givens(self, c, s, k, sgn, w, inverse, dests=None):
        a, b = (k + 1) % 3, (k + 2) % 3
        sg = -sgn if inverse else sgn
        out = [None, None, None]
        if w[a].is_const and w[b].is_const:
            out[a] = self.lin([c, s], [w[a].c, -sg * w[b].c])
            out[b] = self.lin([s, c], [sg * w[a].c, w[b].c])
        else:
            ca = self.mul(c, w[a])
            cb = self.mul(c, w[b])
            sa = self.mul(s, w[a])
            sb = self.mul(s, w[b])
            da = dests[a] if dests else None
            db = dests[b] if dests else None
            out[a] = self.lin([ca, sb], [1.0, -sg], dest=da, scale_free=True)
            out[b] = self.lin([sa, cb], [sg, 1.0], dest=db, scale_free=True)
        out[k] = w[k]
        if dests:
            if dests[a] is not None and (out[a].pl is None
                                         or not self.same_plane(out[a].pl, dests[a])):
                out[a] = self.mov(out[a], dests[a])
            if dests[b] is not None and (out[b].pl is None
                                         or not self.same_plane(out[b].pl, dests[b])):
                out[b] = self.mov(out[b], dests[b])
            if dests[k] is not None and not w[k].is_const:
                out[k] = self.mov(w[k], dests[k])
        return out


# ---------------------------------------------------------------------------
# host-side constants
# ---------------------------------------------------------------------------
def host_consts(rot_fix, trans_fix, joint_axes, mass, com, inertia, damping):
    rot_fix = np.asarray(rot_fix, np.float64)
    trans_fix = np.asarray(trans_fix, np.float64)
    joint_axes = np.asarray(joint_axes, np.float64)
    mass = np.asarray(mass, np.float64)
    com = np.asarray(com, np.float64)
    inertia = np.asarray(inertia, np.float64)
    damping = np.asarray(damping, np.float64)
    C = {}
    C["F"] = [rot_fix[j + 1] for j in range(D)]
    C["p"] = [trans_fix[j + 1] for j in range(D)]
    ax = []
    for j in range(D):
        k = int(np.argmax(np.abs(joint_axes[j])))
        ax.append((k, float(np.sign(joint_axes[j][k]))))
    C["ax"] = ax
    C["m"] = [float(mass[j + 1]) for j in range(D)]
    C["mc"] = [mass[j + 1] * com[j + 1] for j in range(D)]
    Io = []
    for j in range(D):
        cc = com[j + 1]
        cs = np.array([[0, -cc[2], cc[1]], [cc[2], 0, -cc[0]], [-cc[1], cc[0], 0]])
        Io.append(inertia[j + 1] + mass[j + 1] * (cs @ cs.T))
    C["Io"] = Io
    C["damping"] = [float(damping[j]) for j in range(D)]
    C["G"] = 9.81
    return C


# ---------------------------------------------------------------------------
# the physics graph (backend-independent)
# ---------------------------------------------------------------------------
def build_rnea(b: Builder, C):
    Z = VC(0.0)
    vl = [Z, Z, Z]
    va = [Z, Z, Z]
    al = [Z, Z, VC(C["G"])]
    aa = [Z, Z, Z]
    fstore = [[None] * 6 for _ in range(D)]
    for j in range(D):
        F = C["F"][j]
        p = C["p"][j]
        k, sg = C["ax"][j]
        a_, b_ = (k + 1) % 3, (k + 2) % 3
        b.phase = f"fwd{j}"
        s, c = b.sincos(j)
        qd = Val(b.inp("qd", j))
        qdd = Val(b.inp("qdd", j))
        Ft = F.T.tolist()

        def dvec(x, y):
            out = []
            for i in range(3):
                bb, cc = (i + 1) % 3, (i + 2) % 3
                out.append(b.lin([x[i], y[cc], y[bb]], [1.0, -p[bb], p[cc]]))
            return out

        u_vl = b.matvec(Ft, dvec(vl, va))
        u_va = b.matvec(Ft, va)
        u_al = b.matvec(Ft, dvec(al, aa))
        u_aa = b.matvec(Ft, aa)
        std = lambda i: b.state_ap(j, i)
        vl_i = b.givens(c, s, k, sg, u_vl, True, dests=[std(0), std(1), std(2)])
        va_r = b.givens(c, s, k, sg, u_va, True,
                        dests=[std(3 + i) if i != k else None for i in range(3)])
        va_i = list(va_r)
        va_i[k] = b.lin([va_r[k], qd], [1.0, sg], dest=std(3 + k), scale_free=True)
        al_r = b.givens(c, s, k, sg, u_al, True,
                        dests=[std(6 + i) if i == k else None for i in range(3)])
        aa_r = b.givens(c, s, k, sg, u_aa, True)
        aa_i = list(aa_r)
        aa_i[k] = b.lin([aa_r[k], qdd], [1.0, sg], dest=std(9 + k), scale_free=True)
        ek = [0.0, 0.0, 0.0]
        ek[k] = 1.0
        al_i = list(al_r)
        for i in (a_, b_):
            bb, cc = (i + 1) % 3, (i + 2) % 3
            cva = b.lin([va_i[bb], va_i[cc]], [ek[cc], -ek[bb]])
            m1 = b.mul(cva, qd)
            aa_i[i] = b.lin([aa_r[i], m1], [1.0, sg], dest=std(9 + i),
                            scale_free=True)
            cvl = b.lin([vl_i[bb], vl_i[cc]], [ek[cc], -ek[bb]])
            m2 = b.mul(cvl, qd)
            al_i[i] = b.lin([al_r[i], m2], [1.0, sg], dest=std(6 + i),
                            scale_free=True)
        vl, va, al, aa = vl_i, va_i, al_i, aa_i

        # ---- force for this joint ----
        b.phase = f"force{j}"
        m = C["m"][j]
        mc = C["mc"][j].tolist()
        Io = C["Io"][j]
        Iv_l = [b.lin([vl[i], va[(i + 1) % 3], va[(i + 2) % 3]],
                      [m, mc[(i + 2) % 3], -mc[(i + 1) % 3]]) for i in range(3)]
        Ia_l = [b.lin([al[i], aa[(i + 1) % 3], aa[(i + 2) % 3]],
                      [m, mc[(i + 2) % 3], -mc[(i + 1) % 3]]) for i in range(3)]
        Iv_a = [b.lin([va[0], va[1], va[2], vl[(i + 2) % 3], vl[(i + 1) % 3]],
                      [Io[i][0], Io[i][1], Io[i][2],
                       mc[(i + 1) % 3], -mc[(i + 2) % 3]]) for i in range(3)]
        Ia_a = [b.lin([aa[0], aa[1], aa[2], al[(i + 2) % 3], al[(i + 1) % 3]],
                      [Io[i][0], Io[i][1], Io[i][2],
                       mc[(i + 1) % 3], -mc[(i + 2) % 3]]) for i in range(3)]
        cv1 = b.cross(va, Iv_l)
        for i in range(3):
            m1, m2 = cv1[i]
            fstore[j][i] = b.lin([Ia_l[i], m1, m2], [1.0, 1.0, -1.0],
                                 dest=b.f_ap(j, i), scale_free=True)
        cv2 = b.cross(va, Iv_a)
        cv3 = b.cross(vl, Iv_l)
        for i in range(3):
            m1, m2 = cv2[i]
            m3, m4 = cv3[i]
            fstore[j][3 + i] = b.lin([Ia_a[i], m1, m2, m3, m4],
                                     [1.0, 1.0, -1.0, 1.0, -1.0],
                                     dest=b.f_ap(j, 3 + i), scale_free=True)

    # ---- backward pass ----
    cl = [Z, Z, Z]
    ca = [Z, Z, Z]
    for j in range(D - 1, -1, -1):
        F = C["F"][j]
        p = C["p"][j]
        k, sg = C["ax"][j]
        pp = (F.T @ p).tolist()
        b.phase = f"bwd{j}"
        s, c = b.sincos(j)
        f_l = fstore[j][:3]
        f_a = fstore[j][3:]
        tl = [b.lin([f_l[i], cl[i]], [1.0, 1.0]) for i in range(3)]
        ta = [b.lin([f_a[i], ca[i]], [1.0, 1.0]) for i in range(3)]
        b.lin([ta[k], Val(b.inp("qd", j))], [sg, C["damping"][j]],
              dest=b.out_ap(j))
        if j == 0:
            continue
        w_l = b.givens(c, s, k, sg, tl, False)
        w_a = b.givens(c, s, k, sg, ta, False)
        x = []
        for i in range(3):
            bb, cc = (i + 1) % 3, (i + 2) % 3
            x.append(b.lin([w_a[i], w_l[cc], w_l[bb]], [1.0, pp[bb], -pp[cc]]))
        cl = b.matvec(F.tolist(), w_l)
        ca = b.matvec(F.tolist(), x)


# ---------------------------------------------------------------------------
# numpy backend (validation)
# ---------------------------------------------------------------------------
class NumpyBuilder(Builder):
    def __init__(self, q, qd, qdd):
        super().__init__()
        self.q, self.qd, self.qdd = q, qd, qdd
        self.N = q.shape[0]
        self.out = np.zeros((self.N, D), np.float32)
        self._f = {}

    def _w(self, r, dest):
        if dest is not None:
            dest[...] = r
            return dest
        return r

    def _f32(self, x):
        return np.asarray(x, np.float32)

    def p_stt(self, in0, scalar, in1, op1, dest=None, f32=False):
        r = self._f32(in0 * np.float32(scalar))
        if op1 == "add":
            r = self._f32(r + in1)
        elif op1 == "subtract":
            r = self._f32(r - in1)
        else:
            r = self._f32(r * in1)
        return self._w(r, dest)

    def p_tt(self, in0, in1, op, dest=None):
        if op == "mult":
            r = self._f32(in0 * in1)
        elif op == "add":
            r = self._f32(in0 + in1)
        else:
            r = self._f32(in0 - in1)
        return self._w(r, dest)

    def p_affine(self, in0, scale, bias, dest=None):
        return self._w(self._f32(in0 * np.float32(scale) + np.float32(bias)), dest)

    def p_act(self, in0, fname, scale, bias, f32=False):
        z = self._f32(in0) * np.float32(scale) + np.float32(bias)
        if fname == "Copy":
            return self._f32(z)
        if fname == "Abs":
            return self._f32(np.abs(z))
        if fname == "Sin":
            return self._f32(np.sin(z))
        raise ValueError(fname)

    def p_ones(self):
        return np.ones(self.N, np.float32)

    def inp(self, name, j):
        return {"q": self.q, "qd": self.qd, "qdd": self.qdd}[name][:, j].astype(
            np.float32
        )

    def out_ap(self, j):
        return self.out[:, j]

    def f_ap(self, j, i):
        key = (j, i)
        if key not in self._f:
            self._f[key] = np.empty(self.N, np.float32)
        return self._f[key]

    def state_ap(self, j, i):
        return np.empty(self.N, np.float32)


def rnea_numpy(q, qd, qdd, rot_fix, trans_fix, joint_axes, mass, com, inertia,
               damping):
    C = host_consts(rot_fix, trans_fix, joint_axes, mass, com, inertia, damping)
    b = NumpyBuilder(q, qd, qdd)
    build_rnea(b, C)
    return b.out


# ---------------------------------------------------------------------------
# IR backend: records ops on integer-token planes
# ---------------------------------------------------------------------------
class IRBuilder(Builder):
    def __init__(self):
        super().__init__()
        self.ops = []   # (kind, out_token, in_tokens, params, phase)
        self.f32_toks = set()
        self._n = 0
        self.phase = ""

    def _tmp(self, f32=False):
        self._n += 1
        t = ("t", self._n)
        if f32:
            self.f32_toks.add(t)
        return t

    def plane_key(self, pl):
        return pl

    def same_plane(self, a, b):
        return a == b

    def p_stt(self, in0, scalar, in1, op1, dest=None, f32=False):
        out = dest if dest is not None else self._tmp(f32)
        self.ops.append(("stt", out, (in0, in1), (float(scalar), op1),
                         self.phase))
        return out

    def p_tt(self, in0, in1, op, dest=None):
        out = dest if dest is not None else self._tmp()
        self.ops.append(("tt", out, (in0, in1), (op,), self.phase))
        return out

    def p_affine(self, in0, scale, bias, dest=None):
        out = dest if dest is not None else self._tmp()
        self.ops.append(("affine", out, (in0,), (float(scale), float(bias)),
                         self.phase))
        return out

    def p_act(self, in0, fname, scale, bias, f32=False):
        out = self._tmp(f32)
        self.ops.append(("act", out, (in0,), (fname, float(scale), float(bias)),
                         self.phase))
        return out

    def p_ones(self):
        out = ("ones",)
        self.ops.append(("memset", out, (), (1.0,), self.phase))
        return out

    def inp(self, name, j):
        return ("in", name, j)

    def out_ap(self, j):
        return ("out", j)

    def f_ap(self, j, i):
        return ("f", j, i)

    def state_ap(self, j, i):
        return self._tmp()


def dce(ops):
    """drop ops whose results are never used (named 'out' sinks are live)."""
    needed = set()
    keep = [False] * len(ops)
    for idx in range(len(ops) - 1, -1, -1):
        kind, out, ins, params, phase = ops[idx]
        if out[0] == "out" or out in needed:
            keep[idx] = True
            for t in ins:
                needed.add(t)
    return [op for k2, op in zip(keep, ops) if k2]


def ir_stats(ops):
    from collections import Counter

    c = Counter(k for k, *_ in ops)
    last_use = {}
    for idx, (kind, out, ins, params, phase) in enumerate(ops):
        for t in ins:
            if t[0] == "t":
                last_use[t] = idx
    live = set()
    peak = 0
    for idx, (kind, out, ins, params, phase) in enumerate(ops):
        if out[0] == "t":
            live.add(out)
        peak = max(peak, len(live))
        for t in ins:
            if t[0] == "t" and last_use.get(t) == idx:
                live.discard(t)
    return dict(c), peak


# ---------------------------------------------------------------------------
# HEFT-style engine assignment + list schedule
# ---------------------------------------------------------------------------
# engine codes: V = DVE (vector), P = Pool (gpsimd), A = ACT (scalar)
def op_costs(kind, params, f32out, use_gp=True):
    """eligible {engine: cost_ns} for an op. fp16 planes assumed."""
    if kind == "stt":
        return {"V": 691 if f32out else 424}
    if kind == "tt":
        d = {"V": 424}
        if use_gp:
            d["P"] = 1450
        return d
    if kind == "affine":
        # vector tensor_scalar (fp16 4x) or ACT copy
        return {"V": 200, "A": 480}
    if kind == "act":
        return {"A": 600 if f32out else 480}
    if kind == "memset":
        return {"V": 300}
    raise ValueError(kind)


def schedule(ops, f32_toks, use_gp=True):
    """Assign engines and order ops to minimize modeled makespan.
    Returns list of (op, engine)."""
    n = len(ops)
    prod = {}
    for i, (kind, out, ins, params, phase) in enumerate(ops):
        prod[out] = i
    deps = [[] for _ in range(n)]
    succs = [[] for _ in range(n)]
    for i, (kind, out, ins, params, phase) in enumerate(ops):
        seen = set()
        for t in ins:
            j = prod.get(t)
            if j is not None and j not in seen:
                seen.add(j)
                deps[i].append(j)
                succs[j].append(i)
    costs = []
    for (kind, out, ins, params, phase) in ops:
        f32o = out in f32_toks
        costs.append(op_costs(kind, params, f32o, use_gp))
    # upward rank (critical path length to any sink), min-cost weights
    rank = [0.0] * n
    for i in range(n - 1, -1, -1):
        w = min(costs[i].values())
        rank[i] = w + max((rank[s] for s in succs[i]), default=0.0)
    order = sorted(range(n), key=lambda i: -rank[i])
    finish = [0.0] * n
    engine_free = {"V": 0.0, "P": 0.0, "A": 0.0}
    assign = [None] * n
    start = [0.0] * n
    for i in order:
        ready = max((finish[d] for d in deps[i]), default=0.0)
        best = None
        for e, cst in costs[i].items():
            st = max(engine_free[e], ready)
            fin = st + cst
            if best is None or fin < best[0]:
                best = (fin, st, e)
        fin, st, e = best
        assign[i] = e
        start[i] = st
        finish[i] = fin
        engine_free[e] = fin
    # emission order: by start time (stable on original idx). Parents always
    # start strictly before children finish constraints keep this topological,
    # but guard against ties by enforcing dependency order explicitly.
    emit_order = sorted(range(n), key=lambda i: (start[i], i))
    pos = {i: p for p, i in enumerate(emit_order)}
    # fix any topological inversions (possible on ties)
    emitted = []
    done = set()
    pending = list(emit_order)
    import heapq

    indeg = [len(deps[i]) for i in range(n)]
    heap = [(pos[i], i) for i in range(n) if indeg[i] == 0]
    heapq.heapify(heap)
    while heap:
        _, i = heapq.heappop(heap)
        emitted.append(i)
        done.add(i)
        for s in succs[i]:
            indeg[s] -= 1
            if indeg[s] == 0:
                heapq.heappush(heap, (pos[s], s))
    assert len(emitted) == n
    makespan = max(finish)
    busy = {e: sum(costs[i][assign[i]] for i in range(n) if assign[i] == e)
            for e in ("V", "P", "A")}
    return [(ops[i], assign[i]) for i in emitted], makespan, busy


def build_ir(C):
    b = IRBuilder()
    build_rnea(b, C)
    ops = dce(b.ops)
    return ops, b


# ---------------------------------------------------------------------------
# bass emission from IR
# ---------------------------------------------------------------------------
def emit_bass(nc, tc, pools, chunks, out_chunk, sched, f32_toks, fd=FD,
              bench_alias_out=False, dtype16=DT16):
    from concourse import mybir

    f32 = mybir.dt.float32
    fdt = mybir.dt.float16 if dtype16 else mybir.dt.float32
    ALU = {"add": mybir.AluOpType.add, "subtract": mybir.AluOpType.subtract,
           "mult": mybir.AluOpType.mult}
    AFN = {"Copy": mybir.ActivationFunctionType.Copy,
           "Sin": mybir.ActivationFunctionType.Sin,
           "Abs": mybir.ActivationFunctionType.Abs}

    ops = [op for op, e in sched]
    engines = [e for op, e in sched]

    last_use = {}
    for idx, (kind, out, ins, params, phase) in enumerate(ops):
        for t in ins:
            if t[0] == "t":
                last_use[t] = idx

    ftiles = {}
    tmp_ap = {}         # token -> AP
    reg_of = {}         # token -> (pool_name, reg index)
    free_regs = {"reg": [], "reg32": []}
    pend_free = []      # (idx_freed, pool, reg) delayed release
    n_regs = {"reg": 0, "reg32": 0}
    serial = 0
    FREE_DELAY = 24

    def named_ap(tok):
        nonlocal serial
        if tok[0] == "in":
            _, name, j = tok
            v = chunks[name].rearrange("p (f d) -> p d f", d=D)
            return v[:, j, :]
        if tok[0] == "out":
            base = chunks["qdd"] if bench_alias_out else out_chunk
            v = base.rearrange("p (f d) -> p d f", d=D)
            return v[:, tok[1], :]
        if tok[0] == "f":
            _, j, i = tok
            if j not in ftiles:
                serial += 1
                ftiles[j] = pools["fst"].tile([P, 6 * fd], fdt, tag=f"f{j}",
                                              name=f"f{j}", bufs=1)
            t = ftiles[j]
            return t[:, i * fd:(i + 1) * fd]
        if tok[0] == "ones":
            return ones_ap
        raise KeyError(tok)

    def get_ap(tok):
        if tok[0] == "t":
            return tmp_ap[tok]
        return named_ap(tok)

    def alloc_out(tok, idx):
        nonlocal serial
        if tok[0] != "t":
            return named_ap(tok)
        pool = "reg32" if tok in f32_toks else "reg"
        dt = f32 if pool == "reg32" else fdt
        # flush delayed frees
        while pend_free and pend_free[0][0] + FREE_DELAY <= idx:
            _, pl, r = pend_free.pop(0)
            free_regs[pl].append(r)
        if free_regs[pool]:
            r = free_regs[pool].pop()
        else:
            r = n_regs[pool]
            n_regs[pool] += 1
        reg_of[tok] = (pool, r)
        serial += 1
        t = pools[pool].tile([P, fd], dt, tag=f"{pool}{r}", name=f"v{serial}",
                             bufs=1)
        tmp_ap[tok] = t[:, :]
        return tmp_ap[tok]

    def release_ins(ins, idx):
        for t in ins:
            if t[0] == "t" and last_use.get(t) == idx:
                pr = reg_of.pop(t, None)
                if pr is not None:
                    pend_free.append((idx, pr[0], pr[1]))

    ones_ap = None
    eng_count = {"V": 0, "P": 0, "A": 0}
    for idx, (kind, out, ins, params, phase) in enumerate(ops):
        e = engines[idx]
        if kind == "memset":
            serial += 1
            t = pools["misc"].tile([P, fd], fdt, tag="ones", name="ones", bufs=1)
            ones_ap = t[:, :]
            nc.vector.memset(ones_ap, 1.0)
            continue
        out_ap = alloc_out(out, idx)
        eng_count[e] += 1
        if kind == "stt":
            scalar, op1 = params
            nc.vector.scalar_tensor_tensor(out_ap, get_ap(ins[0]), scalar,
                                           get_ap(ins[1]),
                                           mybir.AluOpType.mult, ALU[op1])
        elif kind == "tt":
            eng = nc.gpsimd if e == "P" else nc.vector
            eng.tensor_tensor(out_ap, get_ap(ins[0]), get_ap(ins[1]),
                              ALU[params[0]])
        elif kind == "affine":
            scale, bias = params
            if e == "V":
                nc.vector.tensor_scalar(out_ap, get_ap(ins[0]),
                                        float(scale), mybir.AluOpType.mult,
                                        float(bias), mybir.AluOpType.add)
            else:
                nc.scalar.activation(out_ap, get_ap(ins[0]),
                                     mybir.ActivationFunctionType.Copy,
                                     bias=float(bias), scale=float(scale))
        elif kind == "act":
            fname, scale, bias = params
            nc.scalar.activation(out_ap, get_ap(ins[0]), AFN[fname],
                                 bias=float(bias), scale=float(scale))
        else:
            raise ValueError(kind)
        release_ins(ins, idx)
    return n_regs, eng_count


def _build_nc(C, verbose=False, repeat=1, dtype16=DT16, use_gp=USE_GP):
    import concourse.bacc as bacc
    import concourse.tile as tile_mod
    from concourse import mybir

    ops, bstat = build_ir(C)
    sched, makespan, busy = schedule(ops, bstat.f32_toks, use_gp=use_gp)
    if verbose:
        stats, peak = ir_stats(ops)
        print("IR ops:", stats, "peak live tmps:", peak)
        print("sched makespan model: %.0f us" % (makespan / 1e3),
              "busy(us):", {k: round(v / 1e3) for k, v in busy.items()})

    nc = bacc.Bacc()
    f32 = mybir.dt.float32
    fdt = mybir.dt.float16 if dtype16 else mybir.dt.float32
    # const APs for non-Copy activation biases (Sin bias pi/2 and 0.0, Abs 0.0)
    halfpi = float(HALF_PI)
    _ct = nc.alloc_sbuf_tensor("const-f32-halfpi", [128, 1], f32)
    nc.gpsimd.memset(_ct.ap(), halfpi)
    nc.const_aps.aps[(f32, halfpi)] = _ct.ap()
    nc.all_engine_barrier()
    q_d = nc.dram_tensor("q", [SHARD, D], fdt, kind="ExternalInput")
    qd_d = nc.dram_tensor("qd", [SHARD, D], fdt, kind="ExternalInput")
    qdd_d = nc.dram_tensor("qdd", [SHARD, D], fdt, kind="ExternalInput")
    tau_d = nc.dram_tensor("tau", [SHARD, D], fdt, kind="ExternalOutput")

    with ExitStack() as ctx:
        tc = ctx.enter_context(tile_mod.TileContext(nc))
        io_pool = ctx.enter_context(tc.tile_pool(name="io", bufs=1))
        fst_pool = ctx.enter_context(tc.tile_pool(name="fst", bufs=1))
        reg_pool = ctx.enter_context(tc.tile_pool(name="reg", bufs=1))
        reg32_pool = ctx.enter_context(tc.tile_pool(name="reg32", bufs=1))
        misc_pool = ctx.enter_context(tc.tile_pool(name="misc", bufs=1))
        pools = {"io": io_pool, "fst": fst_pool, "reg": reg_pool,
                 "reg32": reg32_pool, "misc": misc_pool}

        chunks = {}
        for name, dram in (("q", q_d), ("qd", qd_d), ("qdd", qdd_d)):
            t = io_pool.tile([P, D * FD], fdt, tag=f"io_{name}",
                             name=f"ch_{name}", bufs=1)
            nc.sync.dma_start(t[:, :],
                              dram[:, :].rearrange("(p f) d -> p (f d)", p=P))
            chunks[name] = t

        if repeat == 1:
            # out chunk shares the qdd slot (qdd is fully consumed by the
            # forward pass before any tau is written)
            out_chunk = io_pool.tile([P, D * FD], fdt, tag="io_qdd",
                                     name="ch_out", bufs=1)
            n_regs, eng_count = emit_bass(nc, tc, pools, chunks, out_chunk,
                                          sched, bstat.f32_toks,
                                          dtype16=dtype16)
        else:
            # bench mode: tau lands in the qdd chunk itself (timing only)
            out_chunk = chunks["qdd"]
            for _ in range(repeat):
                n_regs, eng_count = emit_bass(nc, tc, pools, chunks, out_chunk,
                                              sched, bstat.f32_toks,
                                              bench_alias_out=True,
                                              dtype16=dtype16)
        if verbose:
            print("registers used:", n_regs, "engine op counts:", eng_count)

        nc.sync.dma_start(tau_d[:, :].rearrange("(p f) d -> p (f d)", p=P),
                          out_chunk[:, :])
    if not nc.is_finalized():
        nc.finalize()
    return nc


def prep_shard_inputs(q, qd, qdd):
    """Cast + shard full inputs into per-core in_maps matching dram dtypes."""
    dt = np.float16 if DT16 else np.float32
    q = np.ascontiguousarray(q, dt)
    qd = np.ascontiguousarray(qd, dt)
    qdd = np.ascontiguousarray(qdd, dt)
    in_maps = []
    for i in range(N_CORES):
        sl = slice(i * SHARD, (i + 1) * SHARD)
        in_maps.append({"q": q[sl], "qd": qd[sl], "qdd": qdd[sl]})
    return in_maps


def kernel(**inputs):
    C = host_consts(inputs["rot_fix"], inputs["trans_fix"], inputs["joint_axes"],
                    inputs["mass"], inputs["com"], inputs["inertia"],
                    inputs["damping"])
    nc = _build_nc(C)

    from concourse.bass_utils import run_bass_kernel_spmd

    in_maps = prep_shard_inputs(inputs["q"], inputs["qd"], inputs["qdd_des"])
    res = run_bass_kernel_spmd(nc, in_maps, list(range(N_CORES)))
    out = np.concatenate([res.results[i]["tau"] for i in range(N_CORES)], 0)
    return out.astype(np.float32)


# revision 5
# speedup vs baseline: 118.8417x; 118.8417x over previous
"""Trainium2 Bass kernel: batched recursive Newton-Euler inverse dynamics
(7-dof serial chain) — data-parallel over 8 NeuronCores.

Per core, the 65536-row shard lives as fp16 planes [128 part, 512 free]
(fp16 halves DVE tensor_tensor time via the 2x_1p perf mode; validated
rel-err ~1.7e-3 vs the 2e-2 gate). Per-link parameters are baked in as
immediate constants. The physics is emitted through a symbolic layer
(Val = a*plane + c) that prunes zeros, folds scales, and chains every
n-term linear combination into n-1 fused scalar_tensor_tensor ops.
Ops are recorded into a tiny IR, dead code is eliminated, and a
HEFT-style list scheduler assigns each op to an engine (DVE / Pool /
ACT) to overlap the three elementwise-capable engines. Trig is computed
once per joint (shared between fwd and bwd passes) with a single range
reduction: s = Sin(z), c = Sin(pi/2 - |z|).
"""

import math
from contextlib import ExitStack

import numpy as np

P = 128
D = 7
N_CORES = 8
BATCH = 524288
SHARD = BATCH // N_CORES      # 65536
FD = SHARD // P               # 512

TWO_PI = 2.0 * math.pi
HALF_PI = math.pi / 2
MAGIC = 12582912.0            # 1.5 * 2**23, fp32 round-to-nearest trick

DT16 = True                   # fp16 planes
USE_GP = True                 # allow Pool (gpsimd) engine for tensor_tensor


# ---------------------------------------------------------------------------
# symbolic value: a * plane + c   (plane None -> pure constant)
# ---------------------------------------------------------------------------
class Val:
    __slots__ = ("pl", "a", "c")

    def __init__(self, pl, a=1.0, c=0.0):
        self.pl = pl
        self.a = float(a)
        self.c = float(c)
        if pl is None:
            self.a = 0.0

    @property
    def is_const(self):
        return self.pl is None or self.a == 0.0


def VC(c):
    return Val(None, 0.0, c)


class Builder:
    """Backend-agnostic emitter. Each primitive is exactly one instruction."""

    def __init__(self):
        self.n_2src = 0
        self.n_1src = 0
        self.n_trig = 0
        self.phase = ""
        self._ones = None
        self._trig = {}

    # ---- primitives (backends) ----
    def p_stt(self, in0, scalar, in1, op1, dest=None, f32=False):
        raise NotImplementedError

    def p_tt(self, in0, in1, op, dest=None):
        raise NotImplementedError

    def p_affine(self, in0, scale, bias, dest=None):
        raise NotImplementedError

    def p_act(self, in0, fname, scale, bias, f32=False):
        raise NotImplementedError

    def p_ones(self):
        raise NotImplementedError

    def inp(self, name, j):
        raise NotImplementedError

    def out_ap(self, j):
        raise NotImplementedError

    def f_ap(self, j, i):
        raise NotImplementedError

    def state_ap(self, j, i):
        raise NotImplementedError

    def plane_key(self, pl):
        return id(pl)

    def same_plane(self, a, b):
        return a is b

    # ---- helpers ----
    def ones(self):
        if self._ones is None:
            self._ones = self.p_ones()
        return self._ones

    def sincos(self, j):
        # one range reduction per joint; cos from |z|: cos z = sin(pi/2 - |z|)
        if j in self._trig:
            return self._trig[j]
        q = self.inp("q", j)
        u = self.p_act(q, "Copy", 1.0 / TWO_PI, MAGIC, f32=True)
        r = self.p_act(u, "Copy", 1.0, -MAGIC, f32=True)
        z = self.p_stt(r, -TWO_PI, q, "add", f32=True)
        s = Val(self.p_act(z, "Sin", 1.0, 0.0))
        a = self.p_act(z, "Abs", 1.0, 0.0, f32=True)
        c = Val(self.p_act(a, "Sin", -1.0, HALF_PI))
        self.n_trig += 2
        self._trig[j] = (s, c)
        return s, c

    def lin(self, vals, coefs, const=0.0, dest=None, exact=False, scale_free=False):
        terms = {}
        c_acc = float(const)
        for v, k in zip(vals, coefs):
            k = float(k)
            if k == 0.0:
                continue
            c_acc += k * v.c
            if v.pl is not None and v.a != 0.0:
                key = self.plane_key(v.pl)
                if key in terms:
                    terms[key][1] += k * v.a
                else:
                    terms[key] = [v.pl, k * v.a]
        tl = [(pl, k) for pl, k in terms.values() if k != 0.0]
        if not tl:
            if dest is not None:
                self.n_1src += 1
                self.p_affine(self.ones(), c_acc, 0.0, dest=dest)
                return Val(dest, 1.0, 0.0)
            return VC(c_acc)
        if c_acc != 0.0:
            tl.append((self.ones(), c_acc))
        if len(tl) == 1:
            pl, k = tl[0]
            if dest is not None:
                self.n_1src += 1
                self.p_affine(pl, k, 0.0, dest=dest)
                return Val(dest, 1.0, 0.0)
            if exact and k != 1.0:
                self.n_1src += 1
                return Val(self.p_affine(pl, k, 0.0), 1.0, 0.0)
            return Val(pl, k, 0.0)
        tl.sort(key=lambda t: abs(t[1]))
        cur_pl, cur_k = tl[0]
        for i in range(1, len(tl)):
            pl_i, k_i = tl[i]
            is_last = i == len(tl) - 1
            use_dest = dest is not None and is_last and (scale_free or k_i == 1.0)
            d = dest if use_dest else None
            self.n_2src += 1
            cur_pl = self.p_stt(cur_pl, cur_k / k_i, pl_i, "add", dest=d)
            cur_k = k_i
        if dest is not None and not self.same_plane(cur_pl, dest):
            self.n_1src += 1
            self.p_affine(cur_pl, cur_k, 0.0, dest=dest)
            return Val(dest, 1.0, 0.0)
        if dest is not None:
            return Val(dest, cur_k if scale_free else 1.0, 0.0)
        if exact and cur_k != 1.0:
            self.n_1src += 1
            return Val(self.p_affine(cur_pl, cur_k, 0.0), 1.0, 0.0)
        return Val(cur_pl, cur_k, 0.0)

    def mov(self, v, dest):
        self.n_1src += 1
        if v.pl is None:
            self.p_affine(self.ones(), v.c, 0.0, dest=dest)
        else:
            self.p_affine(v.pl, v.a, v.c, dest=dest)
        return Val(dest, 1.0, 0.0)

    def mul(self, x, y):
        if x.is_const and y.is_const:
            return VC(x.c * y.c)
        if x.is_const:
            x, y = y, x
        if y.is_const:
            return Val(x.pl, x.a * y.c, x.c * y.c)
        xp, yp = x, y
        if xp.c != 0.0:
            self.n_1src += 1
            xp = Val(self.p_affine(xp.pl, 1.0, xp.c / xp.a), xp.a, 0.0)
        if yp.c != 0.0:
            self.n_1src += 1
            yp = Val(self.p_affine(yp.pl, 1.0, yp.c / yp.a), yp.a, 0.0)
        self.n_2src += 1
        out = self.p_tt(xp.pl, yp.pl, "mult")
        return Val(out, xp.a * yp.a, 0.0)

    def cross(self, u, v):
        out = []
        for i in range(3):
            b, c = (i + 1) % 3, (i + 2) % 3
            m1 = self.mul(u[b], v[c])
            m2 = self.mul(u[c], v[b])
            out.append((m1, m2))
        return out

    def matvec(self, M, v):
        return [self.lin(v, [M[i][0], M[i][1], M[i][2]]) for i in range(3)]

    def givens(self, c, s, k, sgn, w, inverse, dests=None):
        a, b = (k + 1) % 3, (k + 2) % 3
        sg = -sgn if inverse else sgn
        out = [None, None, None]
        if w[a].is_const and w[b].is_const:
            out[a] = self.lin([c, s], [w[a].c, -sg * w[b].c])
            out[b] = self.lin([s, c], [sg * w[a].c, w[b].c])
        else:
            ca = self.mul(c, w[a])
            cb = self.mul(c, w[b])
            sa = self.mul(s, w[a])
            sb = self.mul(s, w[b])
            da = dests[a] if dests else None
            db = dests[b] if dests else None
            out[a] = self.lin([ca, sb], [1.0, -sg], dest=da, scale_free=True)
            out[b] = self.lin([sa, cb], [sg, 1.0], dest=db, scale_free=True)
        out[k] = w[k]
        if dests:
            if dests[a] is not None and (out[a].pl is None
                                         or not self.same_plane(out[a].pl, dests[a])):
                out[a] = self.mov(out[a], dests[a])
            if dests[b] is not None and (out[b].pl is None
                                         or not self.same_plane(out[b].pl, dests[b])):
                out[b] = self.mov(out[b], dests[b])
            if dests[k] is not None and not w[k].is_const:
                out[k] = self.mov(w[k], dests[k])
        return out


# ---------------------------------------------------------------------------
# host-side constants
# ---------------------------------------------------------------------------
def host_consts(rot_fix, trans_fix, joint_axes, mass, com, inertia, damping):
    rot_fix = np.asarray(rot_fix, np.float64)
    trans_fix = np.asarray(trans_fix, np.float64)
    joint_axes = np.asarray(joint_axes, np.float64)
    mass = np.asarray(mass, np.float64)
    com = np.asarray(com, np.float64)
    inertia = np.asarray(inertia, np.float64)
    damping = np.asarray(damping, np.float64)
    C = {}
    C["F"] = [rot_fix[j + 1] for j in range(D)]
    C["p"] = [trans_fix[j + 1] for j in range(D)]
    ax = []
    for j in range(D):
        k = int(np.argmax(np.abs(joint_axes[j])))
        ax.append((k, float(np.sign(joint_axes[j][k]))))
    C["ax"] = ax
    C["m"] = [float(mass[j + 1]) for j in range(D)]
    C["mc"] = [mass[j + 1] * com[j + 1] for j in range(D)]
    Io = []
    for j in range(D):
        cc = com[j + 1]
        cs = np.array([[0, -cc[2], cc[1]], [cc[2], 0, -cc[0]], [-cc[1], cc[0], 0]])
        Io.append(inertia[j + 1] + mass[j + 1] * (cs @ cs.T))
    C["Io"] = Io
    C["damping"] = [float(damping[j]) for j in range(D)]
    C["G"] = 9.81
    return C


# ---------------------------------------------------------------------------
# the physics graph (backend-independent)
# ---------------------------------------------------------------------------
def build_rnea(b: Builder, C):
    Z = VC(0.0)
    vl = [Z, Z, Z]
    va = [Z, Z, Z]
    al = [Z, Z, VC(C["G"])]
    aa = [Z, Z, Z]
    fstore = [[None] * 6 for _ in range(D)]
    for j in range(D):
        F = C["F"][j]
        p = C["p"][j]
        k, sg = C["ax"][j]
        a_, b_ = (k + 1) % 3, (k + 2) % 3
        b.phase = f"fwd{j}"
        s, c = b.sincos(j)
        qd = Val(b.inp("qd", j))
        qdd = Val(b.inp("qdd", j))
        Ft = F.T.tolist()

        def dvec(x, y):
            out = []
            for i in range(3):
                bb, cc = (i + 1) % 3, (i + 2) % 3
                out.append(b.lin([x[i], y[cc], y[bb]], [1.0, -p[bb], p[cc]]))
            return out

        u_vl = b.matvec(Ft, dvec(vl, va))
        u_va = b.matvec(Ft, va)
        u_al = b.matvec(Ft, dvec(al, aa))
        u_aa = b.matvec(Ft, aa)
        std = lambda i: b.state_ap(j, i)
        vl_i = b.givens(c, s, k, sg, u_vl, True, dests=[std(0), std(1), std(2)])
        va_r = b.givens(c, s, k, sg, u_va, True,
                        dests=[std(3 + i) if i != k else None for i in range(3)])
        va_i = list(va_r)
        va_i[k] = b.lin([va_r[k], qd], [1.0, sg], dest=std(3 + k), scale_free=True)
        al_r = b.givens(c, s, k, sg, u_al, True,
                        dests=[std(6 + i) if i == k else None for i in range(3)])
        aa_r = b.givens(c, s, k, sg, u_aa, True)
        aa_i = list(aa_r)
        aa_i[k] = b.lin([aa_r[k], qdd], [1.0, sg], dest=std(9 + k), scale_free=True)
        ek = [0.0, 0.0, 0.0]
        ek[k] = 1.0
        al_i = list(al_r)
        for i in (a_, b_):
            bb, cc = (i + 1) % 3, (i + 2) % 3
            cva = b.lin([va_i[bb], va_i[cc]], [ek[cc], -ek[bb]])
            m1 = b.mul(cva, qd)
            aa_i[i] = b.lin([aa_r[i], m1], [1.0, sg], dest=std(9 + i),
                            scale_free=True)
            cvl = b.lin([vl_i[bb], vl_i[cc]], [ek[cc], -ek[bb]])
            m2 = b.mul(cvl, qd)
            al_i[i] = b.lin([al_r[i], m2], [1.0, sg], dest=std(6 + i),
                            scale_free=True)
        vl, va, al, aa = vl_i, va_i, al_i, aa_i

        # ---- force for this joint ----
        b.phase = f"force{j}"
        m = C["m"][j]
        mc = C["mc"][j].tolist()
        Io = C["Io"][j]
        Iv_l = [b.lin([vl[i], va[(i + 1) % 3], va[(i + 2) % 3]],
                      [m, mc[(i + 2) % 3], -mc[(i + 1) % 3]]) for i in range(3)]
        Ia_l = [b.lin([al[i], aa[(i + 1) % 3], aa[(i + 2) % 3]],
                      [m, mc[(i + 2) % 3], -mc[(i + 1) % 3]]) for i in range(3)]
        Iv_a = [b.lin([va[0], va[1], va[2], vl[(i + 2) % 3], vl[(i + 1) % 3]],
                      [Io[i][0], Io[i][1], Io[i][2],
                       mc[(i + 1) % 3], -mc[(i + 2) % 3]]) for i in range(3)]
        Ia_a = [b.lin([aa[0], aa[1], aa[2], al[(i + 2) % 3], al[(i + 1) % 3]],
                      [Io[i][0], Io[i][1], Io[i][2],
                       mc[(i + 1) % 3], -mc[(i + 2) % 3]]) for i in range(3)]
        cv1 = b.cross(va, Iv_l)
        for i in range(3):
            m1, m2 = cv1[i]
            fstore[j][i] = b.lin([Ia_l[i], m1, m2], [1.0, 1.0, -1.0],
                                 dest=b.f_ap(j, i), scale_free=True)
        cv2 = b.cross(va, Iv_a)
        cv3 = b.cross(vl, Iv_l)
        for i in range(3):
            m1, m2 = cv2[i]
            m3, m4 = cv3[i]
            fstore[j][3 + i] = b.lin([Ia_a[i], m1, m2, m3, m4],
                                     [1.0, 1.0, -1.0, 1.0, -1.0],
                                     dest=b.f_ap(j, 3 + i), scale_free=True)

    # ---- backward pass ----
    cl = [Z, Z, Z]
    ca = [Z, Z, Z]
    for j in range(D - 1, -1, -1):
        F = C["F"][j]
        p = C["p"][j]
        k, sg = C["ax"][j]
        pp = (F.T @ p).tolist()
        b.phase = f"bwd{j}"
        s, c = b.sincos(j)
        f_l = fstore[j][:3]
        f_a = fstore[j][3:]
        tl = [b.lin([f_l[i], cl[i]], [1.0, 1.0]) for i in range(3)]
        ta = [b.lin([f_a[i], ca[i]], [1.0, 1.0]) for i in range(3)]
        b.lin([ta[k], Val(b.inp("qd", j))], [sg, C["damping"][j]],
              dest=b.out_ap(j))
        if j == 0:
            continue
        w_l = b.givens(c, s, k, sg, tl, False)
        w_a = b.givens(c, s, k, sg, ta, False)
        x = []
        for i in range(3):
            bb, cc = (i + 1) % 3, (i + 2) % 3
            x.append(b.lin([w_a[i], w_l[cc], w_l[bb]], [1.0, pp[bb], -pp[cc]]))
        cl = b.matvec(F.tolist(), w_l)
        ca = b.matvec(F.tolist(), x)


# ---------------------------------------------------------------------------
# numpy backend (validation)
# ---------------------------------------------------------------------------
class NumpyBuilder(Builder):
    def __init__(self, q, qd, qdd):
        super().__init__()
        self.q, self.qd, self.qdd = q, qd, qdd
        self.N = q.shape[0]
        self.out = np.zeros((self.N, D), np.float32)
        self._f = {}

    def _w(self, r, dest):
        if dest is not None:
            dest[...] = r
            return dest
        return r

    def _f32(self, x):
        return np.asarray(x, np.float32)

    def p_stt(self, in0, scalar, in1, op1, dest=None, f32=False):
        r = self._f32(in0 * np.float32(scalar))
        if op1 == "add":
            r = self._f32(r + in1)
        elif op1 == "subtract":
            r = self._f32(r - in1)
        else:
            r = self._f32(r * in1)
        return self._w(r, dest)

    def p_tt(self, in0, in1, op, dest=None):
        if op == "mult":
            r = self._f32(in0 * in1)
        elif op == "add":
            r = self._f32(in0 + in1)
        else:
            r = self._f32(in0 - in1)
        return self._w(r, dest)

    def p_affine(self, in0, scale, bias, dest=None):
        return self._w(self._f32(in0 * np.float32(scale) + np.float32(bias)), dest)

    def p_act(self, in0, fname, scale, bias, f32=False):
        z = self._f32(in0) * np.float32(scale) + np.float32(bias)
        if fname == "Copy":
            return self._f32(z)
        if fname == "Abs":
            return self._f32(np.abs(z))
        if fname == "Sin":
            return self._f32(np.sin(z))
        raise ValueError(fname)

    def p_ones(self):
        return np.ones(self.N, np.float32)

    def inp(self, name, j):
        return {"q": self.q, "qd": self.qd, "qdd": self.qdd}[name][:, j].astype(
            np.float32
        )

    def out_ap(self, j):
        return self.out[:, j]

    def f_ap(self, j, i):
        key = (j, i)
        if key not in self._f:
            self._f[key] = np.empty(self.N, np.float32)
        return self._f[key]

    def state_ap(self, j, i):
        return np.empty(self.N, np.float32)


def rnea_numpy(q, qd, qdd, rot_fix, trans_fix, joint_axes, mass, com, inertia,
               damping):
    C = host_consts(rot_fix, trans_fix, joint_axes, mass, com, inertia, damping)
    b = NumpyBuilder(q, qd, qdd)
    build_rnea(b, C)
    return b.out


# ---------------------------------------------------------------------------
# IR backend: records ops on integer-token planes
# ---------------------------------------------------------------------------
class IRBuilder(Builder):
    def __init__(self):
        super().__init__()
        self.ops = []   # (kind, out_token, in_tokens, params, phase)
        self.f32_toks = set()
        self._n = 0
        self.phase = ""

    def _tmp(self, f32=False):
        self._n += 1
        t = ("t", self._n)
        if f32:
            self.f32_toks.add(t)
        return t

    def plane_key(self, pl):
        return pl

    def same_plane(self, a, b):
        return a == b

    def p_stt(self, in0, scalar, in1, op1, dest=None, f32=False):
        out = dest if dest is not None else self._tmp(f32)
        self.ops.append(("stt", out, (in0, in1), (float(scalar), op1),
                         self.phase))
        return out

    def p_tt(self, in0, in1, op, dest=None):
        out = dest if dest is not None else self._tmp()
        self.ops.append(("tt", out, (in0, in1), (op,), self.phase))
        return out

    def p_affine(self, in0, scale, bias, dest=None):
        out = dest if dest is not None else self._tmp()
        self.ops.append(("affine", out, (in0,), (float(scale), float(bias)),
                         self.phase))
        return out

    def p_act(self, in0, fname, scale, bias, f32=False):
        out = self._tmp(f32)
        self.ops.append(("act", out, (in0,), (fname, float(scale), float(bias)),
                         self.phase))
        return out

    def p_ones(self):
        out = ("ones",)
        self.ops.append(("memset", out, (), (1.0,), self.phase))
        return out

    def inp(self, name, j):
        return ("in", name, j)

    def out_ap(self, j):
        return ("out", j)

    def f_ap(self, j, i):
        return ("f", j, i)

    def state_ap(self, j, i):
        return self._tmp()


def dce(ops):
    """drop ops whose results are never used (named 'out' sinks are live)."""
    needed = set()
    keep = [False] * len(ops)
    for idx in range(len(ops) - 1, -1, -1):
        kind, out, ins, params, phase = ops[idx]
        if out[0] == "out" or out in needed:
            keep[idx] = True
            for t in ins:
                needed.add(t)
    return [op for k2, op in zip(keep, ops) if k2]


def ir_stats(ops):
    from collections import Counter

    c = Counter(k for k, *_ in ops)
    last_use = {}
    for idx, (kind, out, ins, params, phase) in enumerate(ops):
        for t in ins:
            if t[0] == "t":
                last_use[t] = idx
    live = set()
    peak = 0
    for idx, (kind, out, ins, params, phase) in enumerate(ops):
        if out[0] == "t":
            live.add(out)
        peak = max(peak, len(live))
        for t in ins:
            if t[0] == "t" and last_use.get(t) == idx:
                live.discard(t)
    return dict(c), peak


# ---------------------------------------------------------------------------
# HEFT-style engine assignment + list schedule
# ---------------------------------------------------------------------------
# engine codes: V = DVE (vector), P = Pool (gpsimd), A = ACT (scalar)
def op_costs(kind, params, f32out, use_gp=True):
    """eligible {engine: cost_ns} for an op. fp16 planes assumed."""
    if kind == "stt":
        return {"V": 691 if f32out else 424}
    if kind == "tt":
        d = {"V": 424}
        if use_gp:
            d["P"] = 1450
        return d
    if kind == "affine":
        # vector tensor_scalar (fp16 4x) or ACT copy
        return {"V": 200, "A": 480}
    if kind == "act":
        return {"A": 600 if f32out else 480}
    if kind == "memset":
        return {"V": 300}
    raise ValueError(kind)


def schedule(ops, f32_toks, use_gp=True):
    """Assign engines and order ops to minimize modeled makespan.
    Returns list of (op, engine)."""
    n = len(ops)
    prod = {}
    for i, (kind, out, ins, params, phase) in enumerate(ops):
        prod[out] = i
    deps = [[] for _ in range(n)]
    succs = [[] for _ in range(n)]
    for i, (kind, out, ins, params, phase) in enumerate(ops):
        seen = set()
        for t in ins:
            j = prod.get(t)
            if j is not None and j not in seen:
                seen.add(j)
                deps[i].append(j)
                succs[j].append(i)
    costs = []
    for (kind, out, ins, params, phase) in ops:
        f32o = out in f32_toks
        costs.append(op_costs(kind, params, f32o, use_gp))
    # upward rank (critical path length to any sink), min-cost weights
    rank = [0.0] * n
    for i in range(n - 1, -1, -1):
        w = min(costs[i].values())
        rank[i] = w + max((rank[s] for s in succs[i]), default=0.0)
    order = sorted(range(n), key=lambda i: -rank[i])
    finish = [0.0] * n
    engine_free = {"V": 0.0, "P": 0.0, "A": 0.0}
    assign = [None] * n
    start = [0.0] * n
    for i in order:
        ready = max((finish[d] for d in deps[i]), default=0.0)
        best = None
        for e, cst in costs[i].items():
            st = max(engine_free[e], ready)
            fin = st + cst
            if best is None or fin < best[0]:
                best = (fin, st, e)
        fin, st, e = best
        assign[i] = e
        start[i] = st
        finish[i] = fin
        engine_free[e] = fin
    # emission order: by start time (stable on original idx). Parents always
    # start strictly before children finish constraints keep this topological,
    # but guard against ties by enforcing dependency order explicitly.
    emit_order = sorted(range(n), key=lambda i: (start[i], i))
    pos = {i: p for p, i in enumerate(emit_order)}
    # fix any topological inversions (possible on ties)
    emitted = []
    done = set()
    pending = list(emit_order)
    import heapq

    indeg = [len(deps[i]) for i in range(n)]
    heap = [(pos[i], i) for i in range(n) if indeg[i] == 0]
    heapq.heapify(heap)
    while heap:
        _, i = heapq.heappop(heap)
        emitted.append(i)
        done.add(i)
        for s in succs[i]:
            indeg[s] -= 1
            if indeg[s] == 0:
                heapq.heappush(heap, (pos[s], s))
    assert len(emitted) == n
    makespan = max(finish)
    busy = {e: sum(costs[i][assign[i]] for i in range(n) if assign[i] == e)
            for e in ("V", "P", "A")}
    return [(ops[i], assign[i]) for i in emitted], makespan, busy


def build_ir(C):
    b = IRBuilder()
    build_rnea(b, C)
    ops = dce(b.ops)
    return ops, b


# ---------------------------------------------------------------------------
# bass emission from IR
# ---------------------------------------------------------------------------
def emit_bass(nc, tc, pools, chunks, out_chunk, sched, f32_toks, fd=FD,
              bench_alias_out=False, dtype16=DT16):
    from concourse import mybir

    f32 = mybir.dt.float32
    fdt = mybir.dt.float16 if dtype16 else mybir.dt.float32
    ALU = {"add": mybir.AluOpType.add, "subtract": mybir.AluOpType.subtract,
           "mult": mybir.AluOpType.mult}
    AFN = {"Copy": mybir.ActivationFunctionType.Copy,
           "Sin": mybir.ActivationFunctionType.Sin,
           "Abs": mybir.ActivationFunctionType.Abs}

    ops = [op for op, e in sched]
    engines = [e for op, e in sched]

    last_use = {}
    for idx, (kind, out, ins, params, phase) in enumerate(ops):
        for t in ins:
            if t[0] == "t":
                last_use[t] = idx

    ftiles = {}
    tmp_ap = {}         # token -> AP
    reg_of = {}         # token -> (pool_name, reg index)
    free_regs = {"reg": [], "reg32": []}
    pend_free = []      # (idx_freed, pool, reg) delayed release
    n_regs = {"reg": 0, "reg32": 0}
    serial = 0
    FREE_DELAY = 6

    def named_ap(tok):
        nonlocal serial
        if tok[0] == "in":
            _, name, j = tok
            v = chunks[name].rearrange("p (f d) -> p d f", d=D)
            return v[:, j, :]
        if tok[0] == "out":
            base = chunks["qdd"] if bench_alias_out else out_chunk
            v = base.rearrange("p (f d) -> p d f", d=D)
            return v[:, tok[1], :]
        if tok[0] == "f":
            _, j, i = tok
            if j not in ftiles:
                serial += 1
                ftiles[j] = pools["fst"].tile([P, 6 * fd], fdt, tag=f"f{j}",
                                              name=f"f{j}", bufs=1)
            t = ftiles[j]
            return t[:, i * fd:(i + 1) * fd]
        if tok[0] == "ones":
            return ones_ap
        raise KeyError(tok)

    def get_ap(tok):
        if tok[0] == "t":
            return tmp_ap[tok]
        return named_ap(tok)

    def alloc_out(tok, idx):
        nonlocal serial
        if tok[0] != "t":
            return named_ap(tok)
        pool = "reg32" if tok in f32_toks else "reg"
        dt = f32 if pool == "reg32" else fdt
        # flush delayed frees
        while pend_free and pend_free[0][0] + FREE_DELAY <= idx:
            _, pl, r = pend_free.pop(0)
            free_regs[pl].append(r)
        if free_regs[pool]:
            r = free_regs[pool].pop()
        else:
            r = n_regs[pool]
            n_regs[pool] += 1
        reg_of[tok] = (pool, r)
        serial += 1
        t = pools[pool].tile([P, fd], dt, tag=f"{pool}{r}", name=f"v{serial}",
                             bufs=1)
        tmp_ap[tok] = t[:, :]
        return tmp_ap[tok]

    def release_ins(ins, idx):
        for t in ins:
            if t[0] == "t" and last_use.get(t) == idx:
                pr = reg_of.pop(t, None)
                if pr is not None:
                    pend_free.append((idx, pr[0], pr[1]))

    ones_ap = None
    eng_count = {"V": 0, "P": 0, "A": 0}
    for idx, (kind, out, ins, params, phase) in enumerate(ops):
        e = engines[idx]
        if kind == "memset":
            serial += 1
            t = pools["misc"].tile([P, fd], fdt, tag="ones", name="ones", bufs=1)
            ones_ap = t[:, :]
            nc.vector.memset(ones_ap, 1.0)
            continue
        out_ap = alloc_out(out, idx)
        eng_count[e] += 1
        if kind == "stt":
            scalar, op1 = params
            nc.vector.scalar_tensor_tensor(out_ap, get_ap(ins[0]), scalar,
                                           get_ap(ins[1]),
                                           mybir.AluOpType.mult, ALU[op1])
        elif kind == "tt":
            eng = nc.gpsimd if e == "P" else nc.vector
            eng.tensor_tensor(out_ap, get_ap(ins[0]), get_ap(ins[1]),
                              ALU[params[0]])
        elif kind == "affine":
            scale, bias = params
            if e == "V":
                nc.vector.tensor_scalar(out_ap, get_ap(ins[0]),
                                        float(scale), float(bias),
                                        mybir.AluOpType.mult,
                                        mybir.AluOpType.add)
            else:
                nc.scalar.activation(out_ap, get_ap(ins[0]),
                                     mybir.ActivationFunctionType.Copy,
                                     bias=float(bias), scale=float(scale))
        elif kind == "act":
            fname, scale, bias = params
            nc.scalar.activation(out_ap, get_ap(ins[0]), AFN[fname],
                                 bias=float(bias), scale=float(scale))
        else:
            raise ValueError(kind)
        release_ins(ins, idx)
    return n_regs, eng_count


def _build_nc(C, verbose=False, repeat=1, dtype16=DT16, use_gp=USE_GP):
    import concourse.bacc as bacc
    import concourse.tile as tile_mod
    from concourse import mybir

    ops, bstat = build_ir(C)
    sched, makespan, busy = schedule(ops, bstat.f32_toks, use_gp=use_gp)
    if verbose:
        stats, peak = ir_stats(ops)
        print("IR ops:", stats, "peak live tmps:", peak)
        print("sched makespan model: %.0f us" % (makespan / 1e3),
              "busy(us):", {k: round(v / 1e3) for k, v in busy.items()})

    nc = bacc.Bacc()
    f32 = mybir.dt.float32
    fdt = mybir.dt.float16 if dtype16 else mybir.dt.float32
    # const APs for non-Copy activation biases (Sin bias pi/2 and 0.0, Abs 0.0)
    halfpi = float(HALF_PI)
    _ct = nc.alloc_sbuf_tensor("const-f32-halfpi", [128, 1], f32)
    nc.gpsimd.memset(_ct.ap(), halfpi)
    nc.const_aps.aps[(f32, halfpi)] = _ct.ap()
    nc.all_engine_barrier()
    q_d = nc.dram_tensor("q", [SHARD, D], fdt, kind="ExternalInput")
    qd_d = nc.dram_tensor("qd", [SHARD, D], fdt, kind="ExternalInput")
    qdd_d = nc.dram_tensor("qdd", [SHARD, D], fdt, kind="ExternalInput")
    tau_d = nc.dram_tensor("tau", [SHARD, D], fdt, kind="ExternalOutput")

    with ExitStack() as ctx:
        tc = ctx.enter_context(tile_mod.TileContext(nc))
        io_pool = ctx.enter_context(tc.tile_pool(name="io", bufs=1))
        fst_pool = ctx.enter_context(tc.tile_pool(name="fst", bufs=1))
        reg_pool = ctx.enter_context(tc.tile_pool(name="reg", bufs=1))
        reg32_pool = ctx.enter_context(tc.tile_pool(name="reg32", bufs=1))
        misc_pool = ctx.enter_context(tc.tile_pool(name="misc", bufs=1))
        pools = {"io": io_pool, "fst": fst_pool, "reg": reg_pool,
                 "reg32": reg32_pool, "misc": misc_pool}

        chunks = {}
        for name, dram in (("q", q_d), ("qd", qd_d), ("qdd", qdd_d)):
            t = io_pool.tile([P, D * FD], fdt, tag=f"io_{name}",
                             name=f"ch_{name}", bufs=1)
            nc.sync.dma_start(t[:, :],
                              dram[:, :].rearrange("(p f) d -> p (f d)", p=P))
            chunks[name] = t

        # tau lands in the qdd chunk (qdd is fully consumed by the forward
        # pass before any tau is written; every tau depends on the full fwd)
        out_chunk = chunks["qdd"]
        for _ in range(repeat):
            n_regs, eng_count = emit_bass(nc, tc, pools, chunks, out_chunk,
                                          sched, bstat.f32_toks,
                                          bench_alias_out=True,
                                          dtype16=dtype16)
        if verbose:
            print("registers used:", n_regs, "engine op counts:", eng_count)

        nc.sync.dma_start(tau_d[:, :].rearrange("(p f) d -> p (f d)", p=P),
                          out_chunk[:, :])
    if not nc.is_finalized():
        nc.finalize()
    return nc


def prep_shard_inputs(q, qd, qdd):
    """Cast + shard full inputs into per-core in_maps matching dram dtypes."""
    dt = np.float16 if DT16 else np.float32
    q = np.ascontiguousarray(q, dt)
    qd = np.ascontiguousarray(qd, dt)
    qdd = np.ascontiguousarray(qdd, dt)
    in_maps = []
    for i in range(N_CORES):
        sl = slice(i * SHARD, (i + 1) * SHARD)
        in_maps.append({"q": q[sl], "qd": qd[sl], "qdd": qdd[sl]})
    return in_maps


def kernel(**inputs):
    C = host_consts(inputs["rot_fix"], inputs["trans_fix"], inputs["joint_axes"],
                    inputs["mass"], inputs["com"], inputs["inertia"],
                    inputs["damping"])
    nc = _build_nc(C)

    from concourse.bass_utils import run_bass_kernel_spmd

    in_maps = prep_shard_inputs(inputs["q"], inputs["qd"], inputs["qdd_des"])
    res = run_bass_kernel_spmd(nc, in_maps, list(range(N_CORES)))
    out = np.concatenate([res.results[i]["tau"] for i in range(N_CORES)], 0)
    return out.astype(np.float32)


# revision 12
# speedup vs baseline: 120.0371x; 1.0101x over previous
"""Trainium2 Bass kernel: batched recursive Newton-Euler inverse dynamics
(7-dof serial chain) — data-parallel over 8 NeuronCores.

Per core, the 65536-row shard lives as fp16 planes [128 part, 512 free]
(fp16 halves DVE tensor_tensor time via the 2x_1p perf mode; validated
rel-err ~1.7e-3 vs the 2e-2 gate). Per-link parameters are baked in as
immediate constants. The physics is emitted through a symbolic layer
(Val = a*plane + c) that prunes zeros, folds scales, and chains every
n-term linear combination into n-1 fused scalar_tensor_tensor ops.
Ops are recorded into a tiny IR, dead code is eliminated, and a
HEFT-style list scheduler assigns each op to an engine (DVE / Pool /
ACT) to overlap the three elementwise-capable engines. Trig is computed
once per joint (shared between fwd and bwd passes) with a single range
reduction: s = Sin(z), c = Sin(pi/2 - |z|).
"""

import math
from contextlib import ExitStack

import numpy as np

P = 128
D = 7
N_CORES = 8
BATCH = 524288
SHARD = BATCH // N_CORES      # 65536
FD = SHARD // P               # 512

TWO_PI = 2.0 * math.pi
HALF_PI = math.pi / 2
MAGIC = 12582912.0            # 1.5 * 2**23, fp32 round-to-nearest trick

DT16 = True                   # fp16 planes
USE_GP = False                # allow Pool (gpsimd) engine for tensor_tensor


# ---------------------------------------------------------------------------
# symbolic value: a * plane + c   (plane None -> pure constant)
# ---------------------------------------------------------------------------
class Val:
    __slots__ = ("pl", "a", "c")

    def __init__(self, pl, a=1.0, c=0.0):
        self.pl = pl
        self.a = float(a)
        self.c = float(c)
        if pl is None:
            self.a = 0.0

    @property
    def is_const(self):
        return self.pl is None or self.a == 0.0


def VC(c):
    return Val(None, 0.0, c)


class Builder:
    """Backend-agnostic emitter. Each primitive is exactly one instruction."""

    def __init__(self):
        self.n_2src = 0
        self.n_1src = 0
        self.n_trig = 0
        self.phase = ""
        self._ones = None
        self._trig = {}

    # ---- primitives (backends) ----
    def p_stt(self, in0, scalar, in1, op1, dest=None, f32=False):
        raise NotImplementedError

    def p_tt(self, in0, in1, op, dest=None):
        raise NotImplementedError

    def p_affine(self, in0, scale, bias, dest=None):
        raise NotImplementedError

    def p_act(self, in0, fname, scale, bias, f32=False):
        raise NotImplementedError

    def p_ones(self):
        raise NotImplementedError

    def inp(self, name, j):
        raise NotImplementedError

    def out_ap(self, j):
        raise NotImplementedError

    def f_ap(self, j, i):
        raise NotImplementedError

    def state_ap(self, j, i):
        raise NotImplementedError

    def plane_key(self, pl):
        return id(pl)

    def same_plane(self, a, b):
        return a is b

    # ---- helpers ----
    def ones(self):
        if self._ones is None:
            self._ones = self.p_ones()
        return self._ones

    def sincos(self, j):
        # one range reduction per joint; cos from |z|: cos z = sin(pi/2 - |z|)
        if j in self._trig:
            return self._trig[j]
        q = self.inp("q", j)
        u = self.p_act(q, "Copy", 1.0 / TWO_PI, MAGIC, f32=True)
        r = self.p_act(u, "Copy", 1.0, -MAGIC, f32=True)
        z = self.p_stt(r, -TWO_PI, q, "add", f32=True)
        s = Val(self.p_act(z, "Sin", 1.0, 0.0))
        a = self.p_act(z, "Abs", 1.0, 0.0, f32=True)
        c = Val(self.p_act(a, "Sin", -1.0, HALF_PI))
        self.n_trig += 2
        self._trig[j] = (s, c)
        return s, c

    def lin(self, vals, coefs, const=0.0, dest=None, exact=False, scale_free=False):
        terms = {}
        c_acc = float(const)
        for v, k in zip(vals, coefs):
            k = float(k)
            if k == 0.0:
                continue
            c_acc += k * v.c
            if v.pl is not None and v.a != 0.0:
                key = self.plane_key(v.pl)
                if key in terms:
                    terms[key][1] += k * v.a
                else:
                    terms[key] = [v.pl, k * v.a]
        tl = [(pl, k) for pl, k in terms.values() if k != 0.0]
        if not tl:
            if dest is not None:
                self.n_1src += 1
                self.p_affine(self.ones(), c_acc, 0.0, dest=dest)
                return Val(dest, 1.0, 0.0)
            return VC(c_acc)
        if c_acc != 0.0:
            tl.append((self.ones(), c_acc))
        if len(tl) == 1:
            pl, k = tl[0]
            if dest is not None:
                self.n_1src += 1
                self.p_affine(pl, k, 0.0, dest=dest)
                return Val(dest, 1.0, 0.0)
            if exact and k != 1.0:
                self.n_1src += 1
                return Val(self.p_affine(pl, k, 0.0), 1.0, 0.0)
            return Val(pl, k, 0.0)
        tl.sort(key=lambda t: abs(t[1]))
        cur_pl, cur_k = tl[0]
        for i in range(1, len(tl)):
            pl_i, k_i = tl[i]
            is_last = i == len(tl) - 1
            use_dest = dest is not None and is_last and (scale_free or k_i == 1.0)
            d = dest if use_dest else None
            self.n_2src += 1
            cur_pl = self.p_stt(cur_pl, cur_k / k_i, pl_i, "add", dest=d)
            cur_k = k_i
        if dest is not None and not self.same_plane(cur_pl, dest):
            self.n_1src += 1
            self.p_affine(cur_pl, cur_k, 0.0, dest=dest)
            return Val(dest, 1.0, 0.0)
        if dest is not None:
            return Val(dest, cur_k if scale_free else 1.0, 0.0)
        if exact and cur_k != 1.0:
            self.n_1src += 1
            return Val(self.p_affine(cur_pl, cur_k, 0.0), 1.0, 0.0)
        return Val(cur_pl, cur_k, 0.0)

    def mov(self, v, dest):
        self.n_1src += 1
        if v.pl is None:
            self.p_affine(self.ones(), v.c, 0.0, dest=dest)
        else:
            self.p_affine(v.pl, v.a, v.c, dest=dest)
        return Val(dest, 1.0, 0.0)

    def mul(self, x, y):
        if x.is_const and y.is_const:
            return VC(x.c * y.c)
        if x.is_const:
            x, y = y, x
        if y.is_const:
            return Val(x.pl, x.a * y.c, x.c * y.c)
        xp, yp = x, y
        if xp.c != 0.0:
            self.n_1src += 1
            xp = Val(self.p_affine(xp.pl, 1.0, xp.c / xp.a), xp.a, 0.0)
        if yp.c != 0.0:
            self.n_1src += 1
            yp = Val(self.p_affine(yp.pl, 1.0, yp.c / yp.a), yp.a, 0.0)
        self.n_2src += 1
        out = self.p_tt(xp.pl, yp.pl, "mult")
        return Val(out, xp.a * yp.a, 0.0)

    def cross(self, u, v):
        out = []
        for i in range(3):
            b, c = (i + 1) % 3, (i + 2) % 3
            m1 = self.mul(u[b], v[c])
            m2 = self.mul(u[c], v[b])
            out.append((m1, m2))
        return out

    def matvec(self, M, v):
        return [self.lin(v, [M[i][0], M[i][1], M[i][2]]) for i in range(3)]

    def givens(self, c, s, k, sgn, w, inverse, dests=None):
        a, b = (k + 1) % 3, (k + 2) % 3
        sg = -sgn if inverse else sgn
        out = [None, None, None]
        if w[a].is_const and w[b].is_const:
            out[a] = self.lin([c, s], [w[a].c, -sg * w[b].c])
            out[b] = self.lin([s, c], [sg * w[a].c, w[b].c])
        else:
            ca = self.mul(c, w[a])
            cb = self.mul(c, w[b])
            sa = self.mul(s, w[a])
            sb = self.mul(s, w[b])
            da = dests[a] if dests else None
            db = dests[b] if dests else None
            out[a] = self.lin([ca, sb], [1.0, -sg], dest=da, scale_free=True)
            out[b] = self.lin([sa, cb], [sg, 1.0], dest=db, scale_free=True)
        out[k] = w[k]
        if dests:
            if dests[a] is not None and (out[a].pl is None
                                         or not self.same_plane(out[a].pl, dests[a])):
                out[a] = self.mov(out[a], dests[a])
            if dests[b] is not None and (out[b].pl is None
                                         or not self.same_plane(out[b].pl, dests[b])):
                out[b] = self.mov(out[b], dests[b])
            if dests[k] is not None and not w[k].is_const:
                out[k] = self.mov(w[k], dests[k])
        return out


# ---------------------------------------------------------------------------
# host-side constants
# ---------------------------------------------------------------------------
def host_consts(rot_fix, trans_fix, joint_axes, mass, com, inertia, damping):
    rot_fix = np.asarray(rot_fix, np.float64)
    trans_fix = np.asarray(trans_fix, np.float64)
    joint_axes = np.asarray(joint_axes, np.float64)
    mass = np.asarray(mass, np.float64)
    com = np.asarray(com, np.float64)
    inertia = np.asarray(inertia, np.float64)
    damping = np.asarray(damping, np.float64)
    C = {}
    C["F"] = [rot_fix[j + 1] for j in range(D)]
    C["p"] = [trans_fix[j + 1] for j in range(D)]
    ax = []
    for j in range(D):
        k = int(np.argmax(np.abs(joint_axes[j])))
        ax.append((k, float(np.sign(joint_axes[j][k]))))
    C["ax"] = ax
    C["m"] = [float(mass[j + 1]) for j in range(D)]
    C["mc"] = [mass[j + 1] * com[j + 1] for j in range(D)]
    Io = []
    for j in range(D):
        cc = com[j + 1]
        cs = np.array([[0, -cc[2], cc[1]], [cc[2], 0, -cc[0]], [-cc[1], cc[0], 0]])
        Io.append(inertia[j + 1] + mass[j + 1] * (cs @ cs.T))
    C["Io"] = Io
    C["damping"] = [float(damping[j]) for j in range(D)]
    C["G"] = 9.81
    return C


# ---------------------------------------------------------------------------
# the physics graph (backend-independent)
# ---------------------------------------------------------------------------
def build_rnea(b: Builder, C):
    Z = VC(0.0)
    vl = [Z, Z, Z]
    va = [Z, Z, Z]
    al = [Z, Z, VC(C["G"])]
    aa = [Z, Z, Z]
    fstore = [[None] * 6 for _ in range(D)]
    for j in range(D):
        F = C["F"][j]
        p = C["p"][j]
        k, sg = C["ax"][j]
        a_, b_ = (k + 1) % 3, (k + 2) % 3
        b.phase = f"fwd{j}"
        s, c = b.sincos(j)
        qd = Val(b.inp("qd", j))
        qdd = Val(b.inp("qdd", j))
        Ft = F.T.tolist()

        def dvec(x, y):
            out = []
            for i in range(3):
                bb, cc = (i + 1) % 3, (i + 2) % 3
                out.append(b.lin([x[i], y[cc], y[bb]], [1.0, -p[bb], p[cc]]))
            return out

        u_vl = b.matvec(Ft, dvec(vl, va))
        u_va = b.matvec(Ft, va)
        u_al = b.matvec(Ft, dvec(al, aa))
        u_aa = b.matvec(Ft, aa)
        std = lambda i: b.state_ap(j, i)
        vl_i = b.givens(c, s, k, sg, u_vl, True, dests=[std(0), std(1), std(2)])
        va_r = b.givens(c, s, k, sg, u_va, True,
                        dests=[std(3 + i) if i != k else None for i in range(3)])
        va_i = list(va_r)
        va_i[k] = b.lin([va_r[k], qd], [1.0, sg], dest=std(3 + k), scale_free=True)
        al_r = b.givens(c, s, k, sg, u_al, True,
                        dests=[std(6 + i) if i == k else None for i in range(3)])
        aa_r = b.givens(c, s, k, sg, u_aa, True)
        aa_i = list(aa_r)
        aa_i[k] = b.lin([aa_r[k], qdd], [1.0, sg], dest=std(9 + k), scale_free=True)
        ek = [0.0, 0.0, 0.0]
        ek[k] = 1.0
        al_i = list(al_r)
        for i in (a_, b_):
            bb, cc = (i + 1) % 3, (i + 2) % 3
            cva = b.lin([va_i[bb], va_i[cc]], [ek[cc], -ek[bb]])
            m1 = b.mul(cva, qd)
            aa_i[i] = b.lin([aa_r[i], m1], [1.0, sg], dest=std(9 + i),
                            scale_free=True)
            cvl = b.lin([vl_i[bb], vl_i[cc]], [ek[cc], -ek[bb]])
            m2 = b.mul(cvl, qd)
            al_i[i] = b.lin([al_r[i], m2], [1.0, sg], dest=std(6 + i),
                            scale_free=True)
        vl, va, al, aa = vl_i, va_i, al_i, aa_i

        # ---- force for this joint ----
        b.phase = f"force{j}"
        m = C["m"][j]
        mc = C["mc"][j].tolist()
        Io = C["Io"][j]
        Iv_l = [b.lin([vl[i], va[(i + 1) % 3], va[(i + 2) % 3]],
                      [m, mc[(i + 2) % 3], -mc[(i + 1) % 3]]) for i in range(3)]
        Ia_l = [b.lin([al[i], aa[(i + 1) % 3], aa[(i + 2) % 3]],
                      [m, mc[(i + 2) % 3], -mc[(i + 1) % 3]]) for i in range(3)]
        Iv_a = [b.lin([va[0], va[1], va[2], vl[(i + 2) % 3], vl[(i + 1) % 3]],
                      [Io[i][0], Io[i][1], Io[i][2],
                       mc[(i + 1) % 3], -mc[(i + 2) % 3]]) for i in range(3)]
        Ia_a = [b.lin([aa[0], aa[1], aa[2], al[(i + 2) % 3], al[(i + 1) % 3]],
                      [Io[i][0], Io[i][1], Io[i][2],
                       mc[(i + 1) % 3], -mc[(i + 2) % 3]]) for i in range(3)]
        cv1 = b.cross(va, Iv_l)
        for i in range(3):
            m1, m2 = cv1[i]
            fstore[j][i] = b.lin([Ia_l[i], m1, m2], [1.0, 1.0, -1.0],
                                 dest=b.f_ap(j, i), scale_free=True)
        cv2 = b.cross(va, Iv_a)
        cv3 = b.cross(vl, Iv_l)
        for i in range(3):
            m1, m2 = cv2[i]
            m3, m4 = cv3[i]
            fstore[j][3 + i] = b.lin([Ia_a[i], m1, m2, m3, m4],
                                     [1.0, 1.0, -1.0, 1.0, -1.0],
                                     dest=b.f_ap(j, 3 + i), scale_free=True)

    # ---- backward pass ----
    cl = [Z, Z, Z]
    ca = [Z, Z, Z]
    for j in range(D - 1, -1, -1):
        F = C["F"][j]
        p = C["p"][j]
        k, sg = C["ax"][j]
        pp = (F.T @ p).tolist()
        b.phase = f"bwd{j}"
        s, c = b.sincos(j)
        f_l = fstore[j][:3]
        f_a = fstore[j][3:]
        tl = [b.lin([f_l[i], cl[i]], [1.0, 1.0]) for i in range(3)]
        ta = [b.lin([f_a[i], ca[i]], [1.0, 1.0]) for i in range(3)]
        b.lin([ta[k], Val(b.inp("qd", j))], [sg, C["damping"][j]],
              dest=b.out_ap(j))
        if j == 0:
            continue
        w_l = b.givens(c, s, k, sg, tl, False)
        w_a = b.givens(c, s, k, sg, ta, False)
        x = []
        for i in range(3):
            bb, cc = (i + 1) % 3, (i + 2) % 3
            x.append(b.lin([w_a[i], w_l[cc], w_l[bb]], [1.0, pp[bb], -pp[cc]]))
        cl = b.matvec(F.tolist(), w_l)
        ca = b.matvec(F.tolist(), x)


# ---------------------------------------------------------------------------
# numpy backend (validation)
# ---------------------------------------------------------------------------
class NumpyBuilder(Builder):
    def __init__(self, q, qd, qdd):
        super().__init__()
        self.q, self.qd, self.qdd = q, qd, qdd
        self.N = q.shape[0]
        self.out = np.zeros((self.N, D), np.float32)
        self._f = {}

    def _w(self, r, dest):
        if dest is not None:
            dest[...] = r
            return dest
        return r

    def _f32(self, x):
        return np.asarray(x, np.float32)

    def p_stt(self, in0, scalar, in1, op1, dest=None, f32=False):
        r = self._f32(in0 * np.float32(scalar))
        if op1 == "add":
            r = self._f32(r + in1)
        elif op1 == "subtract":
            r = self._f32(r - in1)
        else:
            r = self._f32(r * in1)
        return self._w(r, dest)

    def p_tt(self, in0, in1, op, dest=None):
        if op == "mult":
            r = self._f32(in0 * in1)
        elif op == "add":
            r = self._f32(in0 + in1)
        else:
            r = self._f32(in0 - in1)
        return self._w(r, dest)

    def p_affine(self, in0, scale, bias, dest=None):
        return self._w(self._f32(in0 * np.float32(scale) + np.float32(bias)), dest)

    def p_act(self, in0, fname, scale, bias, f32=False):
        z = self._f32(in0) * np.float32(scale) + np.float32(bias)
        if fname == "Copy":
            return self._f32(z)
        if fname == "Abs":
            return self._f32(np.abs(z))
        if fname == "Sin":
            return self._f32(np.sin(z))
        raise ValueError(fname)

    def p_ones(self):
        return np.ones(self.N, np.float32)

    def inp(self, name, j):
        return {"q": self.q, "qd": self.qd, "qdd": self.qdd}[name][:, j].astype(
            np.float32
        )

    def out_ap(self, j):
        return self.out[:, j]

    def f_ap(self, j, i):
        key = (j, i)
        if key not in self._f:
            self._f[key] = np.empty(self.N, np.float32)
        return self._f[key]

    def state_ap(self, j, i):
        return np.empty(self.N, np.float32)


def rnea_numpy(q, qd, qdd, rot_fix, trans_fix, joint_axes, mass, com, inertia,
               damping):
    C = host_consts(rot_fix, trans_fix, joint_axes, mass, com, inertia, damping)
    b = NumpyBuilder(q, qd, qdd)
    build_rnea(b, C)
    return b.out


# ---------------------------------------------------------------------------
# IR backend: records ops on integer-token planes
# ---------------------------------------------------------------------------
class IRBuilder(Builder):
    def __init__(self):
        super().__init__()
        self.ops = []   # (kind, out_token, in_tokens, params, phase)
        self.f32_toks = set()
        self._n = 0
        self.phase = ""

    def _tmp(self, f32=False):
        self._n += 1
        t = ("t", self._n)
        if f32:
            self.f32_toks.add(t)
        return t

    def plane_key(self, pl):
        return pl

    def same_plane(self, a, b):
        return a == b

    def p_stt(self, in0, scalar, in1, op1, dest=None, f32=False):
        out = dest if dest is not None else self._tmp(f32)
        self.ops.append(("stt", out, (in0, in1), (float(scalar), op1),
                         self.phase))
        return out

    def p_tt(self, in0, in1, op, dest=None):
        out = dest if dest is not None else self._tmp()
        self.ops.append(("tt", out, (in0, in1), (op,), self.phase))
        return out

    def p_affine(self, in0, scale, bias, dest=None):
        out = dest if dest is not None else self._tmp()
        self.ops.append(("affine", out, (in0,), (float(scale), float(bias)),
                         self.phase))
        return out

    def p_act(self, in0, fname, scale, bias, f32=False):
        out = self._tmp(f32)
        self.ops.append(("act", out, (in0,), (fname, float(scale), float(bias)),
                         self.phase))
        return out

    def p_ones(self):
        out = ("ones",)
        self.ops.append(("memset", out, (), (1.0,), self.phase))
        return out

    def inp(self, name, j):
        return ("in", name, j)

    def out_ap(self, j):
        return ("out", j)

    def f_ap(self, j, i):
        return ("f", j, i)

    def state_ap(self, j, i):
        return self._tmp()


def dce(ops):
    """drop ops whose results are never used (named 'out' sinks are live)."""
    needed = set()
    keep = [False] * len(ops)
    for idx in range(len(ops) - 1, -1, -1):
        kind, out, ins, params, phase = ops[idx]
        if out[0] == "out" or out in needed:
            keep[idx] = True
            for t in ins:
                needed.add(t)
    return [op for k2, op in zip(keep, ops) if k2]


def ir_stats(ops):
    from collections import Counter

    c = Counter(k for k, *_ in ops)
    last_use = {}
    for idx, (kind, out, ins, params, phase) in enumerate(ops):
        for t in ins:
            if t[0] == "t":
                last_use[t] = idx
    live = set()
    peak = 0
    for idx, (kind, out, ins, params, phase) in enumerate(ops):
        if out[0] == "t":
            live.add(out)
        peak = max(peak, len(live))
        for t in ins:
            if t[0] == "t" and last_use.get(t) == idx:
                live.discard(t)
    return dict(c), peak


# ---------------------------------------------------------------------------
# HEFT-style engine assignment + list schedule
# ---------------------------------------------------------------------------
# engine codes: V = DVE (vector), P = Pool (gpsimd), A = ACT (scalar)
def op_costs(kind, params, f32out, use_gp=True):
    """eligible {engine: cost_ns} for an op. fp16 planes assumed."""
    if kind == "stt":
        return {"V": 691 if f32out else 424}
    if kind == "tt":
        d = {"V": 424}
        if use_gp:
            d["P"] = 1450
        return d
    if kind == "affine":
        # vector tensor_scalar (fp16 4x) or ACT copy
        return {"V": 200, "A": 480}
    if kind == "act":
        return {"A": 600 if f32out else 480}
    if kind == "memset":
        return {"V": 300}
    raise ValueError(kind)


def schedule(ops, f32_toks, use_gp=True):
    """Assign engines and order ops to minimize modeled makespan.
    Returns list of (op, engine)."""
    n = len(ops)
    prod = {}
    for i, (kind, out, ins, params, phase) in enumerate(ops):
        prod[out] = i
    deps = [[] for _ in range(n)]
    succs = [[] for _ in range(n)]
    for i, (kind, out, ins, params, phase) in enumerate(ops):
        seen = set()
        for t in ins:
            j = prod.get(t)
            if j is not None and j not in seen:
                seen.add(j)
                deps[i].append(j)
                succs[j].append(i)
    costs = []
    for (kind, out, ins, params, phase) in ops:
        f32o = out in f32_toks
        costs.append(op_costs(kind, params, f32o, use_gp))
    # upward rank (critical path length to any sink), min-cost weights
    rank = [0.0] * n
    for i in range(n - 1, -1, -1):
        w = min(costs[i].values())
        rank[i] = w + max((rank[s] for s in succs[i]), default=0.0)
    order = sorted(range(n), key=lambda i: -rank[i])
    finish = [0.0] * n
    engine_free = {"V": 0.0, "P": 0.0, "A": 0.0}
    assign = [None] * n
    start = [0.0] * n
    for i in order:
        ready = max((finish[d] for d in deps[i]), default=0.0)
        best = None
        for e, cst in costs[i].items():
            st = max(engine_free[e], ready)
            fin = st + cst
            if best is None or fin < best[0]:
                best = (fin, st, e)
        fin, st, e = best
        assign[i] = e
        start[i] = st
        finish[i] = fin
        engine_free[e] = fin
    # emission order: by start time (stable on original idx). Parents always
    # start strictly before children finish constraints keep this topological,
    # but guard against ties by enforcing dependency order explicitly.
    emit_order = sorted(range(n), key=lambda i: (start[i], i))
    pos = {i: p for p, i in enumerate(emit_order)}
    # fix any topological inversions (possible on ties)
    emitted = []
    done = set()
    pending = list(emit_order)
    import heapq

    indeg = [len(deps[i]) for i in range(n)]
    heap = [(pos[i], i) for i in range(n) if indeg[i] == 0]
    heapq.heapify(heap)
    while heap:
        _, i = heapq.heappop(heap)
        emitted.append(i)
        done.add(i)
        for s in succs[i]:
            indeg[s] -= 1
            if indeg[s] == 0:
                heapq.heappush(heap, (pos[s], s))
    assert len(emitted) == n
    makespan = max(finish)
    busy = {e: sum(costs[i][assign[i]] for i in range(n) if assign[i] == e)
            for e in ("V", "P", "A")}
    return [(ops[i], assign[i]) for i in emitted], makespan, busy


def build_ir(C):
    b = IRBuilder()
    build_rnea(b, C)
    ops = dce(b.ops)
    return ops, b


# ---------------------------------------------------------------------------
# bass emission from IR
# ---------------------------------------------------------------------------
def emit_bass(nc, tc, pools, chunks, out_chunk, sched, f32_toks, fd=FD,
              bench_alias_out=False, dtype16=DT16):
    from concourse import mybir

    f32 = mybir.dt.float32
    fdt = mybir.dt.float16 if dtype16 else mybir.dt.float32
    ALU = {"add": mybir.AluOpType.add, "subtract": mybir.AluOpType.subtract,
           "mult": mybir.AluOpType.mult}
    AFN = {"Copy": mybir.ActivationFunctionType.Copy,
           "Sin": mybir.ActivationFunctionType.Sin,
           "Abs": mybir.ActivationFunctionType.Abs}

    ops = [op for op, e in sched]
    engines = [e for op, e in sched]

    last_use = {}
    for idx, (kind, out, ins, params, phase) in enumerate(ops):
        for t in ins:
            if t[0] == "t":
                last_use[t] = idx

    ftiles = {}
    tmp_ap = {}         # token -> AP
    reg_of = {}         # token -> (pool_name, reg index)
    free_regs = {"reg": [], "reg32": []}
    pend_free = []      # (idx_freed, pool, reg) delayed release
    n_regs = {"reg": 0, "reg32": 0}
    serial = 0
    FREE_DELAY = 6

    def named_ap(tok):
        nonlocal serial
        if tok[0] == "in":
            _, name, j = tok
            v = chunks[name].rearrange("p (d f) -> p d f", d=D)
            return v[:, j, :]
        if tok[0] == "out":
            base = chunks["qdd"] if bench_alias_out else out_chunk
            v = base.rearrange("p (d f) -> p d f", d=D)
            return v[:, tok[1], :]
        if tok[0] == "f":
            _, j, i = tok
            if j not in ftiles:
                serial += 1
                ftiles[j] = pools["fst"].tile([P, 6 * fd], fdt, tag=f"f{j}",
                                              name=f"f{j}", bufs=1)
            t = ftiles[j]
            return t[:, i * fd:(i + 1) * fd]
        if tok[0] == "ones":
            return ones_ap
        raise KeyError(tok)

    def get_ap(tok):
        if tok[0] == "t":
            return tmp_ap[tok]
        return named_ap(tok)

    def alloc_out(tok, idx):
        nonlocal serial
        if tok[0] != "t":
            return named_ap(tok)
        pool = "reg32" if tok in f32_toks else "reg"
        dt = f32 if pool == "reg32" else fdt
        # flush delayed frees
        while pend_free and pend_free[0][0] + FREE_DELAY <= idx:
            _, pl, r = pend_free.pop(0)
            free_regs[pl].append(r)
        if free_regs[pool]:
            r = free_regs[pool].pop()
        else:
            r = n_regs[pool]
            n_regs[pool] += 1
        reg_of[tok] = (pool, r)
        serial += 1
        t = pools[pool].tile([P, fd], dt, tag=f"{pool}{r}", name=f"v{serial}",
                             bufs=1)
        tmp_ap[tok] = t[:, :]
        return tmp_ap[tok]

    def release_ins(ins, idx):
        for t in ins:
            if t[0] == "t" and last_use.get(t) == idx:
                pr = reg_of.pop(t, None)
                if pr is not None:
                    pend_free.append((idx, pr[0], pr[1]))

    ones_ap = None
    eng_count = {"V": 0, "P": 0, "A": 0}
    for idx, (kind, out, ins, params, phase) in enumerate(ops):
        e = engines[idx]
        if kind == "memset":
            serial += 1
            t = pools["misc"].tile([P, fd], fdt, tag="ones", name="ones", bufs=1)
            ones_ap = t[:, :]
            nc.vector.memset(ones_ap, 1.0)
            continue
        out_ap = alloc_out(out, idx)
        eng_count[e] += 1
        if kind == "stt":
            scalar, op1 = params
            nc.vector.scalar_tensor_tensor(out_ap, get_ap(ins[0]), scalar,
                                           get_ap(ins[1]),
                                           mybir.AluOpType.mult, ALU[op1])
        elif kind == "tt":
            eng = nc.gpsimd if e == "P" else nc.vector
            eng.tensor_tensor(out_ap, get_ap(ins[0]), get_ap(ins[1]),
                              ALU[params[0]])
        elif kind == "affine":
            scale, bias = params
            if e == "V":
                nc.vector.tensor_scalar(out_ap, get_ap(ins[0]),
                                        float(scale), float(bias),
                                        mybir.AluOpType.mult,
                                        mybir.AluOpType.add)
            else:
                nc.scalar.activation(out_ap, get_ap(ins[0]),
                                     mybir.ActivationFunctionType.Copy,
                                     bias=float(bias), scale=float(scale))
        elif kind == "act":
            fname, scale, bias = params
            nc.scalar.activation(out_ap, get_ap(ins[0]), AFN[fname],
                                 bias=float(bias), scale=float(scale))
        else:
            raise ValueError(kind)
        release_ins(ins, idx)
    return n_regs, eng_count


def _build_nc(C, verbose=False, repeat=1, dtype16=DT16, use_gp=USE_GP):
    import concourse.bacc as bacc
    import concourse.tile as tile_mod
    from concourse import mybir

    ops, bstat = build_ir(C)
    sched, makespan, busy = schedule(ops, bstat.f32_toks, use_gp=use_gp)
    if verbose:
        stats, peak = ir_stats(ops)
        print("IR ops:", stats, "peak live tmps:", peak)
        print("sched makespan model: %.0f us" % (makespan / 1e3),
              "busy(us):", {k: round(v / 1e3) for k, v in busy.items()})

    nc = bacc.Bacc()
    f32 = mybir.dt.float32
    fdt = mybir.dt.float16 if dtype16 else mybir.dt.float32
    # const APs for non-Copy activation biases (Sin bias pi/2 and 0.0, Abs 0.0)
    halfpi = float(HALF_PI)
    _ct = nc.alloc_sbuf_tensor("const-f32-halfpi", [128, 1], f32)
    nc.gpsimd.memset(_ct.ap(), halfpi)
    nc.const_aps.aps[(f32, halfpi)] = _ct.ap()
    nc.all_engine_barrier()
    # planar [D, SHARD] layout (host pre-transposes): every plane is a
    # contiguous stride-1 AP, so fp16 ops hit the 2x_1p perf mode
    q_d = nc.dram_tensor("q", [D, SHARD], fdt, kind="ExternalInput")
    qd_d = nc.dram_tensor("qd", [D, SHARD], fdt, kind="ExternalInput")
    qdd_d = nc.dram_tensor("qdd", [D, SHARD], fdt, kind="ExternalInput")
    tau_d = nc.dram_tensor("tau", [D, SHARD], fdt, kind="ExternalOutput")

    with ExitStack() as ctx:
        tc = ctx.enter_context(tile_mod.TileContext(nc))
        io_pool = ctx.enter_context(tc.tile_pool(name="io", bufs=1))
        fst_pool = ctx.enter_context(tc.tile_pool(name="fst", bufs=1))
        reg_pool = ctx.enter_context(tc.tile_pool(name="reg", bufs=1))
        reg32_pool = ctx.enter_context(tc.tile_pool(name="reg32", bufs=1))
        misc_pool = ctx.enter_context(tc.tile_pool(name="misc", bufs=1))
        pools = {"io": io_pool, "fst": fst_pool, "reg": reg_pool,
                 "reg32": reg32_pool, "misc": misc_pool}

        chunks = {}
        for name, dram in (("q", q_d), ("qd", qd_d), ("qdd", qdd_d)):
            t = io_pool.tile([P, D * FD], fdt, tag=f"io_{name}",
                             name=f"ch_{name}", bufs=1)
            nc.sync.dma_start(t[:, :],
                              dram[:, :].rearrange("d (p f) -> p (d f)", p=P))
            chunks[name] = t

        # tau lands in the qdd chunk (qdd is fully consumed by the forward
        # pass before any tau is written; every tau depends on the full fwd)
        out_chunk = chunks["qdd"]
        for _ in range(repeat):
            n_regs, eng_count = emit_bass(nc, tc, pools, chunks, out_chunk,
                                          sched, bstat.f32_toks,
                                          bench_alias_out=True,
                                          dtype16=dtype16)
        if verbose:
            print("registers used:", n_regs, "engine op counts:", eng_count)

        nc.sync.dma_start(tau_d[:, :].rearrange("d (p f) -> p (d f)", p=P),
                          out_chunk[:, :])
    if not nc.is_finalized():
        nc.finalize()
    return nc


def prep_shard_inputs(q, qd, qdd):
    """Cast + shard + pre-transpose full inputs into per-core in_maps
    matching the planar [D, SHARD] dram layout."""
    dt = np.float16 if DT16 else np.float32
    in_maps = []
    for i in range(N_CORES):
        sl = slice(i * SHARD, (i + 1) * SHARD)
        in_maps.append({
            "q": np.ascontiguousarray(np.asarray(q)[sl].T, dt),
            "qd": np.ascontiguousarray(np.asarray(qd)[sl].T, dt),
            "qdd": np.ascontiguousarray(np.asarray(qdd)[sl].T, dt),
        })
    return in_maps


def unpack_tau(tau_core):
    """[D, SHARD] per-core output -> [SHARD, D] float32."""
    return np.asarray(tau_core).T.astype(np.float32)


def kernel(**inputs):
    C = host_consts(inputs["rot_fix"], inputs["trans_fix"], inputs["joint_axes"],
                    inputs["mass"], inputs["com"], inputs["inertia"],
                    inputs["damping"])
    nc = _build_nc(C)

    from concourse.bass_utils import run_bass_kernel_spmd

    in_maps = prep_shard_inputs(inputs["q"], inputs["qd"], inputs["qdd_des"])
    res = run_bass_kernel_spmd(nc, in_maps, list(range(N_CORES)))
    out = np.concatenate([unpack_tau(res.results[i]["tau"])
                          for i in range(N_CORES)], 0)
    return out


# revision 22
# speedup vs baseline: 206.8388x; 1.7231x over previous
"""Trainium2 Bass kernel: batched recursive Newton-Euler inverse dynamics
(7-dof serial chain) — data-parallel over 8 NeuronCores.

Per core, the 65536-row shard lives as fp16 planes [128 part, 512 free]
(fp16 halves DVE tensor_tensor time via the 2x_1p perf mode; validated
rel-err ~1.7e-3 vs the 2e-2 gate). Per-link parameters are baked in as
immediate constants. The physics is emitted through a symbolic layer
(Val = a*plane + c) that prunes zeros, folds scales, and chains every
n-term linear combination into n-1 fused scalar_tensor_tensor ops.
Ops are recorded into a tiny IR, dead code is eliminated, and a
HEFT-style list scheduler assigns each op to an engine (DVE / Pool /
ACT) to overlap the three elementwise-capable engines. Trig is computed
once per joint (shared between fwd and bwd passes) with a single range
reduction: s = Sin(z), c = Sin(pi/2 - |z|).
"""

import math
from contextlib import ExitStack

import numpy as np

P = 128
D = 7
N_CORES = 8
BATCH = 524288
SHARD = BATCH // N_CORES      # 65536
FD = SHARD // P               # 512

TWO_PI = 2.0 * math.pi
HALF_PI = math.pi / 2
MAGIC = 12582912.0            # 1.5 * 2**23, fp32 round-to-nearest trick

DT16 = True                   # fp16 planes
USE_GP = False                # allow Pool (gpsimd) engine for tensor_tensor
TRIG_CACHE = True             # keep fwd sin/cos planes alive for bwd reuse
                              # (costs ~14 live regs; disable to fit fp32)


# ---------------------------------------------------------------------------
# symbolic value: a * plane + c   (plane None -> pure constant)
# ---------------------------------------------------------------------------
class Val:
    __slots__ = ("pl", "a", "c")

    def __init__(self, pl, a=1.0, c=0.0):
        self.pl = pl
        self.a = float(a)
        self.c = float(c)
        if pl is None:
            self.a = 0.0

    @property
    def is_const(self):
        return self.pl is None or self.a == 0.0


def VC(c):
    return Val(None, 0.0, c)


class Builder:
    """Backend-agnostic emitter. Each primitive is exactly one instruction."""

    def __init__(self):
        self.n_2src = 0
        self.n_1src = 0
        self.n_trig = 0
        self.phase = ""
        self._ones = None
        self._trig = {}

    # ---- primitives (backends) ----
    def p_stt(self, in0, scalar, in1, op1, dest=None, f32=False):
        raise NotImplementedError

    def p_tt(self, in0, in1, op, dest=None):
        raise NotImplementedError

    def p_affine(self, in0, scale, bias, dest=None):
        raise NotImplementedError

    def p_act(self, in0, fname, scale, bias, f32=False):
        raise NotImplementedError

    def p_ones(self):
        raise NotImplementedError

    def inp(self, name, j):
        raise NotImplementedError

    def out_ap(self, j):
        raise NotImplementedError

    def f_ap(self, j, i):
        raise NotImplementedError

    def state_ap(self, j, i):
        raise NotImplementedError

    def plane_key(self, pl):
        return id(pl)

    def same_plane(self, a, b):
        return a is b

    # ---- helpers ----
    def ones(self):
        if self._ones is None:
            self._ones = self.p_ones()
        return self._ones

    def sincos(self, j):
        # one range reduction per joint; cos from |z|: cos z = sin(pi/2 - |z|)
        if j in self._trig:
            return self._trig[j]
        q = self.inp("q", j)
        u = self.p_act(q, "Copy", 1.0 / TWO_PI, MAGIC, f32=True)
        r = self.p_act(u, "Copy", 1.0, -MAGIC, f32=True)
        z = self.p_stt(r, -TWO_PI, q, "add", f32=True)
        s = Val(self.p_act(z, "Sin", 1.0, 0.0))
        a = self.p_act(z, "Abs", 1.0, 0.0, f32=True)
        c = Val(self.p_act(a, "Sin", -1.0, HALF_PI))
        self.n_trig += 2
        if TRIG_CACHE:
            self._trig[j] = (s, c)
        return s, c

    def lin(self, vals, coefs, const=0.0, dest=None, exact=False, scale_free=False):
        terms = {}
        c_acc = float(const)
        for v, k in zip(vals, coefs):
            k = float(k)
            if k == 0.0:
                continue
            c_acc += k * v.c
            if v.pl is not None and v.a != 0.0:
                key = self.plane_key(v.pl)
                if key in terms:
                    terms[key][1] += k * v.a
                else:
                    terms[key] = [v.pl, k * v.a]
        tl = [(pl, k) for pl, k in terms.values() if k != 0.0]
        if not tl:
            if dest is not None:
                self.n_1src += 1
                self.p_affine(self.ones(), c_acc, 0.0, dest=dest)
                return Val(dest, 1.0, 0.0)
            return VC(c_acc)
        if c_acc != 0.0:
            tl.append((self.ones(), c_acc))
        if len(tl) == 1:
            pl, k = tl[0]
            if dest is not None:
                self.n_1src += 1
                self.p_affine(pl, k, 0.0, dest=dest)
                return Val(dest, 1.0, 0.0)
            if exact and k != 1.0:
                self.n_1src += 1
                return Val(self.p_affine(pl, k, 0.0), 1.0, 0.0)
            return Val(pl, k, 0.0)
        tl.sort(key=lambda t: abs(t[1]))
        cur_pl, cur_k = tl[0]
        for i in range(1, len(tl)):
            pl_i, k_i = tl[i]
            is_last = i == len(tl) - 1
            use_dest = dest is not None and is_last and (scale_free or k_i == 1.0)
            d = dest if use_dest else None
            self.n_2src += 1
            cur_pl = self.p_stt(cur_pl, cur_k / k_i, pl_i, "add", dest=d)
            cur_k = k_i
        if dest is not None and not self.same_plane(cur_pl, dest):
            self.n_1src += 1
            self.p_affine(cur_pl, cur_k, 0.0, dest=dest)
            return Val(dest, 1.0, 0.0)
        if dest is not None:
            return Val(dest, cur_k if scale_free else 1.0, 0.0)
        if exact and cur_k != 1.0:
            self.n_1src += 1
            return Val(self.p_affine(cur_pl, cur_k, 0.0), 1.0, 0.0)
        return Val(cur_pl, cur_k, 0.0)

    def mov(self, v, dest):
        self.n_1src += 1
        if v.pl is None:
            self.p_affine(self.ones(), v.c, 0.0, dest=dest)
        else:
            self.p_affine(v.pl, v.a, v.c, dest=dest)
        return Val(dest, 1.0, 0.0)

    def mul(self, x, y):
        if x.is_const and y.is_const:
            return VC(x.c * y.c)
        if x.is_const:
            x, y = y, x
        if y.is_const:
            return Val(x.pl, x.a * y.c, x.c * y.c)
        xp, yp = x, y
        if xp.c != 0.0:
            self.n_1src += 1
            xp = Val(self.p_affine(xp.pl, 1.0, xp.c / xp.a), xp.a, 0.0)
        if yp.c != 0.0:
            self.n_1src += 1
            yp = Val(self.p_affine(yp.pl, 1.0, yp.c / yp.a), yp.a, 0.0)
        self.n_2src += 1
        out = self.p_tt(xp.pl, yp.pl, "mult")
        return Val(out, xp.a * yp.a, 0.0)

    def cross(self, u, v):
        out = []
        for i in range(3):
            b, c = (i + 1) % 3, (i + 2) % 3
            m1 = self.mul(u[b], v[c])
            m2 = self.mul(u[c], v[b])
            out.append((m1, m2))
        return out

    def matvec(self, M, v):
        return [self.lin(v, [M[i][0], M[i][1], M[i][2]]) for i in range(3)]

    def givens(self, c, s, k, sgn, w, inverse, dests=None):
        a, b = (k + 1) % 3, (k + 2) % 3
        sg = -sgn if inverse else sgn
        out = [None, None, None]
        if w[a].is_const and w[b].is_const:
            out[a] = self.lin([c, s], [w[a].c, -sg * w[b].c])
            out[b] = self.lin([s, c], [sg * w[a].c, w[b].c])
        else:
            ca = self.mul(c, w[a])
            cb = self.mul(c, w[b])
            sa = self.mul(s, w[a])
            sb = self.mul(s, w[b])
            da = dests[a] if dests else None
            db = dests[b] if dests else None
            out[a] = self.lin([ca, sb], [1.0, -sg], dest=da, scale_free=True)
            out[b] = self.lin([sa, cb], [sg, 1.0], dest=db, scale_free=True)
        out[k] = w[k]
        if dests:
            if dests[a] is not None and (out[a].pl is None
                                         or not self.same_plane(out[a].pl, dests[a])):
                out[a] = self.mov(out[a], dests[a])
            if dests[b] is not None and (out[b].pl is None
                                         or not self.same_plane(out[b].pl, dests[b])):
                out[b] = self.mov(out[b], dests[b])
            if dests[k] is not None and not w[k].is_const:
                out[k] = self.mov(w[k], dests[k])
        return out


# ---------------------------------------------------------------------------
# host-side constants
# ---------------------------------------------------------------------------
def host_consts(rot_fix, trans_fix, joint_axes, mass, com, inertia, damping):
    rot_fix = np.asarray(rot_fix, np.float64)
    trans_fix = np.asarray(trans_fix, np.float64)
    joint_axes = np.asarray(joint_axes, np.float64)
    mass = np.asarray(mass, np.float64)
    com = np.asarray(com, np.float64)
    inertia = np.asarray(inertia, np.float64)
    damping = np.asarray(damping, np.float64)
    C = {}
    C["F"] = [rot_fix[j + 1] for j in range(D)]
    C["p"] = [trans_fix[j + 1] for j in range(D)]
    ax = []
    for j in range(D):
        k = int(np.argmax(np.abs(joint_axes[j])))
        ax.append((k, float(np.sign(joint_axes[j][k]))))
    C["ax"] = ax
    C["m"] = [float(mass[j + 1]) for j in range(D)]
    C["mc"] = [mass[j + 1] * com[j + 1] for j in range(D)]
    Io = []
    for j in range(D):
        cc = com[j + 1]
        cs = np.array([[0, -cc[2], cc[1]], [cc[2], 0, -cc[0]], [-cc[1], cc[0], 0]])
        Io.append(inertia[j + 1] + mass[j + 1] * (cs @ cs.T))
    C["Io"] = Io
    C["damping"] = [float(damping[j]) for j in range(D)]
    C["G"] = 9.81
    return C


# ---------------------------------------------------------------------------
# the physics graph (backend-independent)
# ---------------------------------------------------------------------------
def build_rnea(b: Builder, C):
    Z = VC(0.0)
    vl = [Z, Z, Z]
    va = [Z, Z, Z]
    al = [Z, Z, VC(C["G"])]
    aa = [Z, Z, Z]
    fstore = [[None] * 6 for _ in range(D)]
    for j in range(D):
        F = C["F"][j]
        p = C["p"][j]
        k, sg = C["ax"][j]
        a_, b_ = (k + 1) % 3, (k + 2) % 3
        b.phase = f"fwd{j}"
        s, c = b.sincos(j)
        qd = Val(b.inp("qd", j))
        qdd = Val(b.inp("qdd", j))
        Ft = F.T.tolist()

        def dvec(x, y):
            out = []
            for i in range(3):
                bb, cc = (i + 1) % 3, (i + 2) % 3
                out.append(b.lin([x[i], y[cc], y[bb]], [1.0, -p[bb], p[cc]]))
            return out

        u_vl = b.matvec(Ft, dvec(vl, va))
        u_va = b.matvec(Ft, va)
        u_al = b.matvec(Ft, dvec(al, aa))
        u_aa = b.matvec(Ft, aa)
        std = lambda i: b.state_ap(j, i)
        vl_i = b.givens(c, s, k, sg, u_vl, True, dests=[std(0), std(1), std(2)])
        va_r = b.givens(c, s, k, sg, u_va, True,
                        dests=[std(3 + i) if i != k else None for i in range(3)])
        va_i = list(va_r)
        va_i[k] = b.lin([va_r[k], qd], [1.0, sg], dest=std(3 + k), scale_free=True)
        al_r = b.givens(c, s, k, sg, u_al, True,
                        dests=[std(6 + i) if i == k else None for i in range(3)])
        aa_r = b.givens(c, s, k, sg, u_aa, True)
        aa_i = list(aa_r)
        aa_i[k] = b.lin([aa_r[k], qdd], [1.0, sg], dest=std(9 + k), scale_free=True)
        ek = [0.0, 0.0, 0.0]
        ek[k] = 1.0
        al_i = list(al_r)
        for i in (a_, b_):
            bb, cc = (i + 1) % 3, (i + 2) % 3
            cva = b.lin([va_i[bb], va_i[cc]], [ek[cc], -ek[bb]])
            m1 = b.mul(cva, qd)
            aa_i[i] = b.lin([aa_r[i], m1], [1.0, sg], dest=std(9 + i),
                            scale_free=True)
            cvl = b.lin([vl_i[bb], vl_i[cc]], [ek[cc], -ek[bb]])
            m2 = b.mul(cvl, qd)
            al_i[i] = b.lin([al_r[i], m2], [1.0, sg], dest=std(6 + i),
                            scale_free=True)
        vl, va, al, aa = vl_i, va_i, al_i, aa_i

        # ---- force for this joint ----
        b.phase = f"force{j}"
        m = C["m"][j]
        mc = C["mc"][j].tolist()
        Io = C["Io"][j]
        Iv_l = [b.lin([vl[i], va[(i + 1) % 3], va[(i + 2) % 3]],
                      [m, mc[(i + 2) % 3], -mc[(i + 1) % 3]]) for i in range(3)]
        Ia_l = [b.lin([al[i], aa[(i + 1) % 3], aa[(i + 2) % 3]],
                      [m, mc[(i + 2) % 3], -mc[(i + 1) % 3]]) for i in range(3)]
        Iv_a = [b.lin([va[0], va[1], va[2], vl[(i + 2) % 3], vl[(i + 1) % 3]],
                      [Io[i][0], Io[i][1], Io[i][2],
                       mc[(i + 1) % 3], -mc[(i + 2) % 3]]) for i in range(3)]
        Ia_a = [b.lin([aa[0], aa[1], aa[2], al[(i + 2) % 3], al[(i + 1) % 3]],
                      [Io[i][0], Io[i][1], Io[i][2],
                       mc[(i + 1) % 3], -mc[(i + 2) % 3]]) for i in range(3)]
        cv1 = b.cross(va, Iv_l)
        for i in range(3):
            m1, m2 = cv1[i]
            fstore[j][i] = b.lin([Ia_l[i], m1, m2], [1.0, 1.0, -1.0],
                                 dest=b.f_ap(j, i), scale_free=True)
        cv2 = b.cross(va, Iv_a)
        cv3 = b.cross(vl, Iv_l)
        for i in range(3):
            m1, m2 = cv2[i]
            m3, m4 = cv3[i]
            fstore[j][3 + i] = b.lin([Ia_a[i], m1, m2, m3, m4],
                                     [1.0, 1.0, -1.0, 1.0, -1.0],
                                     dest=b.f_ap(j, 3 + i), scale_free=True)

    # ---- backward pass ----
    cl = [Z, Z, Z]
    ca = [Z, Z, Z]
    for j in range(D - 1, -1, -1):
        F = C["F"][j]
        p = C["p"][j]
        k, sg = C["ax"][j]
        pp = (F.T @ p).tolist()
        b.phase = f"bwd{j}"
        s, c = b.sincos(j)
        f_l = fstore[j][:3]
        f_a = fstore[j][3:]
        tl = [b.lin([f_l[i], cl[i]], [1.0, 1.0]) for i in range(3)]
        ta = [b.lin([f_a[i], ca[i]], [1.0, 1.0]) for i in range(3)]
        b.lin([ta[k], Val(b.inp("qd", j))], [sg, C["damping"][j]],
              dest=b.out_ap(j))
        if j == 0:
            continue
        w_l = b.givens(c, s, k, sg, tl, False)
        w_a = b.givens(c, s, k, sg, ta, False)
        x = []
        for i in range(3):
            bb, cc = (i + 1) % 3, (i + 2) % 3
            x.append(b.lin([w_a[i], w_l[cc], w_l[bb]], [1.0, pp[bb], -pp[cc]]))
        cl = b.matvec(F.tolist(), w_l)
        ca = b.matvec(F.tolist(), x)


# ---------------------------------------------------------------------------
# numpy backend (validation)
# ---------------------------------------------------------------------------
class NumpyBuilder(Builder):
    def __init__(self, q, qd, qdd):
        super().__init__()
        self.q, self.qd, self.qdd = q, qd, qdd
        self.N = q.shape[0]
        self.out = np.zeros((self.N, D), np.float32)
        self._f = {}

    def _w(self, r, dest):
        if dest is not None:
            dest[...] = r
            return dest
        return r

    def _f32(self, x):
        return np.asarray(x, np.float32)

    def p_stt(self, in0, scalar, in1, op1, dest=None, f32=False):
        r = self._f32(in0 * np.float32(scalar))
        if op1 == "add":
            r = self._f32(r + in1)
        elif op1 == "subtract":
            r = self._f32(r - in1)
        else:
            r = self._f32(r * in1)
        return self._w(r, dest)

    def p_tt(self, in0, in1, op, dest=None):
        if op == "mult":
            r = self._f32(in0 * in1)
        elif op == "add":
            r = self._f32(in0 + in1)
        else:
            r = self._f32(in0 - in1)
        return self._w(r, dest)

    def p_affine(self, in0, scale, bias, dest=None):
        return self._w(self._f32(in0 * np.float32(scale) + np.float32(bias)), dest)

    def p_act(self, in0, fname, scale, bias, f32=False):
        z = self._f32(in0) * np.float32(scale) + np.float32(bias)
        if fname == "Copy":
            return self._f32(z)
        if fname == "Abs":
            return self._f32(np.abs(z))
        if fname == "Sin":
            return self._f32(np.sin(z))
        raise ValueError(fname)

    def p_ones(self):
        return np.ones(self.N, np.float32)

    def inp(self, name, j):
        return {"q": self.q, "qd": self.qd, "qdd": self.qdd}[name][:, j].astype(
            np.float32
        )

    def out_ap(self, j):
        return self.out[:, j]

    def f_ap(self, j, i):
        key = (j, i)
        if key not in self._f:
            self._f[key] = np.empty(self.N, np.float32)
        return self._f[key]

    def state_ap(self, j, i):
        return np.empty(self.N, np.float32)


def rnea_numpy(q, qd, qdd, rot_fix, trans_fix, joint_axes, mass, com, inertia,
               damping):
    C = host_consts(rot_fix, trans_fix, joint_axes, mass, com, inertia, damping)
    b = NumpyBuilder(q, qd, qdd)
    build_rnea(b, C)
    return b.out


# ---------------------------------------------------------------------------
# IR backend: records ops on integer-token planes
# ---------------------------------------------------------------------------
class IRBuilder(Builder):
    def __init__(self):
        super().__init__()
        self.ops = []   # (kind, out_token, in_tokens, params, phase)
        self.f32_toks = set()
        self._n = 0
        self.phase = ""

    def _tmp(self, f32=False):
        self._n += 1
        t = ("t", self._n)
        if f32:
            self.f32_toks.add(t)
        return t

    def plane_key(self, pl):
        return pl

    def same_plane(self, a, b):
        return a == b

    def p_stt(self, in0, scalar, in1, op1, dest=None, f32=False):
        out = dest if dest is not None else self._tmp(f32)
        self.ops.append(("stt", out, (in0, in1), (float(scalar), op1),
                         self.phase))
        return out

    def p_tt(self, in0, in1, op, dest=None):
        out = dest if dest is not None else self._tmp()
        self.ops.append(("tt", out, (in0, in1), (op,), self.phase))
        return out

    def p_affine(self, in0, scale, bias, dest=None):
        out = dest if dest is not None else self._tmp()
        self.ops.append(("affine", out, (in0,), (float(scale), float(bias)),
                         self.phase))
        return out

    def p_act(self, in0, fname, scale, bias, f32=False):
        out = self._tmp(f32)
        self.ops.append(("act", out, (in0,), (fname, float(scale), float(bias)),
                         self.phase))
        return out

    def p_ones(self):
        out = ("ones",)
        self.ops.append(("memset", out, (), (1.0,), self.phase))
        return out

    def inp(self, name, j):
        return ("in", name, j)

    def out_ap(self, j):
        return ("out", j)

    def f_ap(self, j, i):
        return ("f", j, i)

    def state_ap(self, j, i):
        return self._tmp()


def dce(ops):
    """drop ops whose results are never used (named 'out' sinks are live)."""
    needed = set()
    keep = [False] * len(ops)
    for idx in range(len(ops) - 1, -1, -1):
        kind, out, ins, params, phase = ops[idx]
        if out[0] == "out" or out in needed:
            keep[idx] = True
            for t in ins:
                needed.add(t)
    return [op for k2, op in zip(keep, ops) if k2]


def ir_stats(ops):
    from collections import Counter

    c = Counter(k for k, *_ in ops)
    last_use = {}
    for idx, (kind, out, ins, params, phase) in enumerate(ops):
        for t in ins:
            if t[0] == "t":
                last_use[t] = idx
    live = set()
    peak = 0
    for idx, (kind, out, ins, params, phase) in enumerate(ops):
        if out[0] == "t":
            live.add(out)
        peak = max(peak, len(live))
        for t in ins:
            if t[0] == "t" and last_use.get(t) == idx:
                live.discard(t)
    return dict(c), peak


# ---------------------------------------------------------------------------
# HEFT-style engine assignment + list schedule
# ---------------------------------------------------------------------------
# engine codes: V = DVE (vector), P = Pool (gpsimd), A = ACT (scalar)
def op_costs(kind, params, f32out, use_gp=True):
    """eligible {engine: cost_ns} for an op. fp16 planes assumed."""
    if kind == "stt":
        return {"V": 691 if f32out else 424}
    if kind == "tt":
        d = {"V": 424}
        if use_gp:
            d["P"] = 1450
        return d
    if kind == "affine":
        # vector tensor_scalar (fp16 4x) or ACT copy
        return {"V": 200, "A": 480}
    if kind == "act":
        return {"A": 600 if f32out else 480}
    if kind == "memset":
        return {"V": 300}
    raise ValueError(kind)


def interleave(ops, window=8):
    """Topological reorder that avoids scheduling an op directly after the op
    that produced one of its inputs (the DVE pays an SBUF read-after-write
    bubble between dependent back-to-back instructions). Picks among the
    first `window` ready ops in original order."""
    import heapq

    n = len(ops)
    prod = {}
    for i, (kind, out, ins, params, phase) in enumerate(ops):
        prod[out] = i
    succs = [[] for _ in range(n)]
    ndeps = [0] * n
    for i, (kind, out, ins, params, phase) in enumerate(ops):
        seen = set()
        for t in ins:
            j = prod.get(t)
            if j is not None and j not in seen:
                seen.add(j)
                succs[j].append(i)
                ndeps[i] += 1
    ready = [i for i in range(n) if ndeps[i] == 0]
    heapq.heapify(ready)
    order = []
    last_out = None
    while ready:
        cand = heapq.nsmallest(window, ready)
        pick = None
        for i in cand:
            if last_out is None or last_out not in ops[i][2]:
                pick = i
                break
        if pick is None:
            pick = cand[0]
        ready.remove(pick)
        heapq.heapify(ready)
        order.append(pick)
        last_out = ops[pick][1]
        for s in succs[pick]:
            ndeps[s] -= 1
            if ndeps[s] == 0:
                heapq.heappush(ready, s)
    assert len(order) == n
    return [ops[i] for i in order]


def schedule_simple(ops, f32_toks):
    """Program-order emission (with RAW-bubble interleave); engines as the
    baseline: stt/tt/memset -> V, affine/act -> A."""
    ops = interleave(ops)
    eng = {"stt": "V", "tt": "V", "memset": "V", "affine": "A", "act": "A"}
    sched = [(op, eng[op[0]]) for op in ops]
    busy = {"V": sum(1 for op in ops if eng[op[0]] == "V"),
            "A": sum(1 for op in ops if eng[op[0]] == "A"), "P": 0}
    return sched, 0.0, busy


def schedule(ops, f32_toks, use_gp=True):
    """Assign engines and order ops to minimize modeled makespan.
    Returns list of (op, engine)."""
    n = len(ops)
    prod = {}
    for i, (kind, out, ins, params, phase) in enumerate(ops):
        prod[out] = i
    deps = [[] for _ in range(n)]
    succs = [[] for _ in range(n)]
    for i, (kind, out, ins, params, phase) in enumerate(ops):
        seen = set()
        for t in ins:
            j = prod.get(t)
            if j is not None and j not in seen:
                seen.add(j)
                deps[i].append(j)
                succs[j].append(i)
    costs = []
    for (kind, out, ins, params, phase) in ops:
        f32o = out in f32_toks
        costs.append(op_costs(kind, params, f32o, use_gp))
    # upward rank (critical path length to any sink), min-cost weights
    rank = [0.0] * n
    for i in range(n - 1, -1, -1):
        w = min(costs[i].values())
        rank[i] = w + max((rank[s] for s in succs[i]), default=0.0)
    order = sorted(range(n), key=lambda i: -rank[i])
    finish = [0.0] * n
    engine_free = {"V": 0.0, "P": 0.0, "A": 0.0}
    assign = [None] * n
    start = [0.0] * n
    for i in order:
        ready = max((finish[d] for d in deps[i]), default=0.0)
        best = None
        for e, cst in costs[i].items():
            st = max(engine_free[e], ready)
            fin = st + cst
            if best is None or fin < best[0]:
                best = (fin, st, e)
        fin, st, e = best
        assign[i] = e
        start[i] = st
        finish[i] = fin
        engine_free[e] = fin
    # emission order: by start time (stable on original idx). Parents always
    # start strictly before children finish constraints keep this topological,
    # but guard against ties by enforcing dependency order explicitly.
    emit_order = sorted(range(n), key=lambda i: (start[i], i))
    pos = {i: p for p, i in enumerate(emit_order)}
    # fix any topological inversions (possible on ties)
    emitted = []
    done = set()
    pending = list(emit_order)
    import heapq

    indeg = [len(deps[i]) for i in range(n)]
    heap = [(pos[i], i) for i in range(n) if indeg[i] == 0]
    heapq.heapify(heap)
    while heap:
        _, i = heapq.heappop(heap)
        emitted.append(i)
        done.add(i)
        for s in succs[i]:
            indeg[s] -= 1
            if indeg[s] == 0:
                heapq.heappush(heap, (pos[s], s))
    assert len(emitted) == n
    makespan = max(finish)
    busy = {e: sum(costs[i][assign[i]] for i in range(n) if assign[i] == e)
            for e in ("V", "P", "A")}
    return [(ops[i], assign[i]) for i in emitted], makespan, busy


def build_ir(C):
    b = IRBuilder()
    build_rnea(b, C)
    ops = dce(b.ops)
    return ops, b


# ---------------------------------------------------------------------------
# bass emission from IR
# ---------------------------------------------------------------------------
def emit_bass(nc, tc, pools, chunks, out_chunk, sched, f32_toks, fd=FD,
              bench_alias_out=False, dtype16=DT16):
    from concourse import mybir

    f32 = mybir.dt.float32
    fdt = mybir.dt.float16 if dtype16 else mybir.dt.float32
    ALU = {"add": mybir.AluOpType.add, "subtract": mybir.AluOpType.subtract,
           "mult": mybir.AluOpType.mult}
    AFN = {"Copy": mybir.ActivationFunctionType.Copy,
           "Sin": mybir.ActivationFunctionType.Sin,
           "Abs": mybir.ActivationFunctionType.Abs}

    ops = [op for op, e in sched]
    engines = [e for op, e in sched]

    last_use = {}
    for idx, (kind, out, ins, params, phase) in enumerate(ops):
        for t in ins:
            if t[0] == "t":
                last_use[t] = idx

    ftiles = {}
    tmp_ap = {}         # token -> AP
    reg_of = {}         # token -> (pool_name, reg index)
    free_regs = {"reg": [], "reg32": []}
    pend_free = []      # (idx_freed, pool, reg) delayed release
    n_regs = {"reg": 0, "reg32": 0}
    serial = 0
    FREE_DELAY = 0

    def named_ap(tok):
        nonlocal serial
        if tok[0] == "in":
            _, name, j = tok
            v = chunks[name].rearrange("p (d f) -> p d f", d=D)
            return v[:, j, :]
        if tok[0] == "out":
            base = chunks["qdd"] if bench_alias_out else out_chunk
            v = base.rearrange("p (d f) -> p d f", d=D)
            return v[:, tok[1], :]
        if tok[0] == "f":
            _, j, i = tok
            if j not in ftiles:
                serial += 1
                ftiles[j] = pools["fst"].tile([P, 6 * fd], fdt, tag=f"f{j}",
                                              name=f"f{j}", bufs=1)
            t = ftiles[j]
            return t[:, i * fd:(i + 1) * fd]
        if tok[0] == "ones":
            return ones_ap
        raise KeyError(tok)

    def get_ap(tok):
        if tok[0] == "t":
            return tmp_ap[tok]
        return named_ap(tok)

    def alloc_out(tok, idx):
        nonlocal serial
        if tok[0] != "t":
            return named_ap(tok)
        pool = "reg32" if tok in f32_toks else "reg"
        dt = f32 if pool == "reg32" else fdt
        # flush delayed frees
        while pend_free and pend_free[0][0] + FREE_DELAY <= idx:
            _, pl, r = pend_free.pop(0)
            free_regs[pl].append(r)
        if free_regs[pool]:
            r = free_regs[pool].pop()
        else:
            r = n_regs[pool]
            n_regs[pool] += 1
        reg_of[tok] = (pool, r)
        serial += 1
        t = pools[pool].tile([P, fd], dt, tag=f"{pool}{r}", name=f"v{serial}",
                             bufs=1)
        tmp_ap[tok] = t[:, :]
        return tmp_ap[tok]

    def release_ins(ins, idx):
        for t in ins:
            if t[0] == "t" and last_use.get(t) == idx:
                pr = reg_of.pop(t, None)
                if pr is not None:
                    pend_free.append((idx, pr[0], pr[1]))

    ones_ap = None
    eng_count = {"V": 0, "P": 0, "A": 0}
    for idx, (kind, out, ins, params, phase) in enumerate(ops):
        e = engines[idx]
        if kind == "memset":
            serial += 1
            t = pools["misc"].tile([P, fd], fdt, tag="ones", name="ones", bufs=1)
            ones_ap = t[:, :]
            nc.vector.memset(ones_ap, 1.0)
            continue
        out_ap = alloc_out(out, idx)
        eng_count[e] += 1
        if kind == "stt":
            scalar, op1 = params
            nc.vector.scalar_tensor_tensor(out_ap, get_ap(ins[0]), scalar,
                                           get_ap(ins[1]),
                                           mybir.AluOpType.mult, ALU[op1])
        elif kind == "tt":
            eng = nc.gpsimd if e == "P" else nc.vector
            eng.tensor_tensor(out_ap, get_ap(ins[0]), get_ap(ins[1]),
                              ALU[params[0]])
        elif kind == "affine":
            scale, bias = params
            if e == "V":
                nc.vector.tensor_scalar(out_ap, get_ap(ins[0]),
                                        float(scale), float(bias),
                                        mybir.AluOpType.mult,
                                        mybir.AluOpType.add)
            else:
                nc.scalar.activation(out_ap, get_ap(ins[0]),
                                     mybir.ActivationFunctionType.Copy,
                                     bias=float(bias), scale=float(scale))
        elif kind == "act":
            fname, scale, bias = params
            nc.scalar.activation(out_ap, get_ap(ins[0]), AFN[fname],
                                 bias=float(bias), scale=float(scale))
        else:
            raise ValueError(kind)
        release_ins(ins, idx)
    return n_regs, eng_count


def _build_nc(C, verbose=False, repeat=1, dtype16=DT16, use_gp=USE_GP,
              heft=False):
    import concourse.bacc as bacc
    import concourse.tile as tile_mod
    from concourse import mybir

    ops, bstat = build_ir(C)
    if heft:
        sched, makespan, busy = schedule(ops, bstat.f32_toks, use_gp=use_gp)
    else:
        sched, makespan, busy = schedule_simple(ops, bstat.f32_toks)
    if verbose:
        stats, peak = ir_stats(ops)
        print("IR ops:", stats, "peak live tmps:", peak)
        print("sched makespan model: %.0f us" % (makespan / 1e3),
              "busy(us):", {k: round(v / 1e3) for k, v in busy.items()})

    nc = bacc.Bacc()
    f32 = mybir.dt.float32
    fdt = mybir.dt.float16 if dtype16 else mybir.dt.float32
    # const APs for non-Copy activation biases (Sin bias pi/2 and 0.0, Abs 0.0)
    halfpi = float(HALF_PI)
    _ct = nc.alloc_sbuf_tensor("const-f32-halfpi", [128, 1], f32)
    nc.gpsimd.memset(_ct.ap(), halfpi)
    nc.const_aps.aps[(f32, halfpi)] = _ct.ap()
    nc.all_engine_barrier()
    # host pre-arranges inputs to the exact SBUF tile layout [P, D*FD]
    # (plane-major per partition): every plane is a contiguous stride-1 AP,
    # so fp16 ops hit the 2x_1p perf mode, and the DMA is a straight copy
    q_d = nc.dram_tensor("q", [P, D * FD], fdt, kind="ExternalInput")
    qd_d = nc.dram_tensor("qd", [P, D * FD], fdt, kind="ExternalInput")
    qdd_d = nc.dram_tensor("qdd", [P, D * FD], fdt, kind="ExternalInput")
    tau_d = nc.dram_tensor("tau", [P, D * FD], fdt, kind="ExternalOutput")

    with ExitStack() as ctx:
        tc = ctx.enter_context(tile_mod.TileContext(nc))
        io_pool = ctx.enter_context(tc.tile_pool(name="io", bufs=1))
        fst_pool = ctx.enter_context(tc.tile_pool(name="fst", bufs=1))
        reg_pool = ctx.enter_context(tc.tile_pool(name="reg", bufs=1))
        reg32_pool = ctx.enter_context(tc.tile_pool(name="reg32", bufs=1))
        misc_pool = ctx.enter_context(tc.tile_pool(name="misc", bufs=1))
        pools = {"io": io_pool, "fst": fst_pool, "reg": reg_pool,
                 "reg32": reg32_pool, "misc": misc_pool}

        chunks = {}
        for name, dram in (("q", q_d), ("qd", qd_d), ("qdd", qdd_d)):
            t = io_pool.tile([P, D * FD], fdt, tag=f"io_{name}",
                             name=f"ch_{name}", bufs=1)
            nc.sync.dma_start(t[:, :], dram[:, :])
            chunks[name] = t

        # tau lands in the qdd chunk (qdd is fully consumed by the forward
        # pass before any tau is written; every tau depends on the full fwd)
        out_chunk = chunks["qdd"]
        for _ in range(repeat):
            n_regs, eng_count = emit_bass(nc, tc, pools, chunks, out_chunk,
                                          sched, bstat.f32_toks,
                                          bench_alias_out=True,
                                          dtype16=dtype16)
        if verbose:
            print("registers used:", n_regs, "engine op counts:", eng_count)

        nc.sync.dma_start(tau_d[:, :], out_chunk[:, :])
    if not nc.is_finalized():
        nc.finalize()
    return nc


def _pack(a, dt):
    # [SHARD, D] rows -> [P, D*FD] plane-major per partition
    return np.ascontiguousarray(
        np.asarray(a).reshape(P, FD, D).transpose(0, 2, 1).reshape(P, D * FD),
        dt)


def prep_shard_inputs(q, qd, qdd, dt=None):
    """Cast + shard + pre-arrange full inputs into per-core in_maps
    matching the plane-major [P, D*FD] dram layout."""
    if dt is None:
        dt = np.float16 if DT16 else np.float32
    in_maps = []
    for i in range(N_CORES):
        sl = slice(i * SHARD, (i + 1) * SHARD)
        in_maps.append({
            "q": _pack(np.asarray(q)[sl], dt),
            "qd": _pack(np.asarray(qd)[sl], dt),
            "qdd": _pack(np.asarray(qdd)[sl], dt),
        })
    return in_maps


def unpack_tau(tau_core):
    """[P, D*FD] per-core output -> [SHARD, D] float32."""
    a = np.asarray(tau_core).reshape(P, D, FD).transpose(0, 2, 1)
    return np.ascontiguousarray(a.reshape(SHARD, D)).astype(np.float32)


def kernel(**inputs):
    C = host_consts(inputs["rot_fix"], inputs["trans_fix"], inputs["joint_axes"],
                    inputs["mass"], inputs["com"], inputs["inertia"],
                    inputs["damping"])
    nc = _build_nc(C)

    from concourse.bass_utils import run_bass_kernel_spmd

    in_maps = prep_shard_inputs(inputs["q"], inputs["qd"], inputs["qdd_des"])
    res = run_bass_kernel_spmd(nc, in_maps, list(range(N_CORES)))
    out = np.concatenate([unpack_tau(res.results[i]["tau"])
                          for i in range(N_CORES)], 0)
    return out
